# revision 39
# baseline (speedup 1.0000x reference)
"""Trainium2 Bass kernel for nn_DecoderMinLSTMGNN.

Model (per sample): two MinLSTM layers (D=512) over T=4096 steps, residual,
LayerNorm, projection D->1.  B=8 samples are data-parallel across the 8
NeuronCores (one sample per core).  344.5us baseline -> ~240us.

Layout is channels-major: x^T [D, T].  The time-dim linear recurrence
h_t = a_t*h_{t-1} + (1-a_t)*htilde_t runs on the VectorE TensorTensorScan
(one independent recurrence per partition, scanned along the free dim).

Key optimizations:
- h-gate bias elimination (g = h - bh substitution): the recurrence becomes
  bias-free (init -bh); the bias folds into the next layer's f/i gate
  biases (bf1_eff = bf1 + Wf1 @ bh0) and into the LN/projection stats via
  an extra lhsT column (c = bh1_eff) + host-side constants.  Removes all
  64 bias matmuls.
- fp8 (e4m3) DoubleRow matmuls for the f/i gate projections (half the
  cycles/row of bf16): weights are scaled by 64 into fp8's normal range
  and the scale is undone for free via the sigmoid's input-scale field.
  Gate noise is strongly damped (sigmoid slope, a in (0,1), contractive
  scan), so fp8 costs <1e-3 of final rel-err.  The h-gate (htilde path)
  and the LN stats stay bf16; PSUM and scan state are fp32.
- ScalarE act-table phase batching: sigmoid and reciprocal live in
  different act-table sets (1.3us per reload; naive interleave costs ~97
  loads).  Work is organized in (layer, half-of-T) phases: 32 sigmoids,
  then 4 wide reciprocals.  Two artificial-dependency tricks keep the
  scheduler from interleaving phases, both numerically exact:
    min-gate:  next phase's biases pass through min(bias, r) with
               |bias| < 0.45 <= 0.5 <= r = 1/(f+i);
    max-gate:  den[:,0:1] = max(den[:,0:1], i_last) since den = f+i >= i.
  Result: 9 table loads total.
- Deferred scans: each phase's 4 wide scans (the serial DVE hub) are
  emitted after the NEXT sig phase so the priority-heap scheduler uses
  them as gap filler instead of queueing dens behind them.
- bf16 everywhere else (DVE 2x tensor-tensor mode, half SBUF/DMA) and
  wide [128,1024/2048] instructions to amortize per-instruction overhead.
- SBUF slot aliasing (f/r share a ring, den/a share a ring, g2 reuses the
  g1 half-0 slots, gf8 reuses the xf8 slots) to fit in 208KB/partition.
- Epilogue: res/square/stats matmuls interleaved per channel-group with
  the layer-2 scans; LN + projection collapse into 3 matmul-accumulated
  row-stats (s1, s3 = wg.res, sc = c.res, s2 = res^2) + a short fp32
  epilogue on [8,512] tiles.
"""

import numpy as np
import ml_dtypes

import concourse.bass as bass
import concourse.mybir as mybir
import concourse.tile as tile
from concourse.bass_utils import run_bass_kernel_spmd

F32 = mybir.dt.float32
BF16 = mybir.dt.bfloat16
FP8 = mybir.dt.float8e4
DR = mybir.MatmulPerfMode.DoubleRow
WS = 64.0
AF = mybir.ActivationFunctionType
OP = mybir.AluOpType

B, T, D = 8, 4096, 512
OUT = 1
LN_EPS = 1e-5
TT = 512                 # time-tile size
NT = T // TT             # 8 time tiles
G = D // 128             # 4 channel groups
K = D // 128             # 4 contraction chunks
TPH = 4                  # time tiles per phase (half)
HALF = TPH * TT          # 2048
NH = NT // TPH           # 2 halves

MAX_WAITS = 1


def _split_excess_waits(nc):
    """walrus in this container rejects >1 semaphore wait per instruction
    ("Too many sync wait commands"); move excess waits onto NoOps."""
    for fn in nc.m.functions:
        for bb in fn.blocks:
            new_list = []
            changed = False
            for inst in bb.instructions:
                si = inst.sync_info
                waits = list(si.on_wait) if si is not None and si.on_wait else []
                if len(waits) > MAX_WAITS:
                    changed = True
                    overflow = waits[:-MAX_WAITS]
                    si.on_wait = waits[-MAX_WAITS:]
                    for j in range(0, len(overflow), MAX_WAITS):
                        new_list.append(mybir.InstNoOp(
                            name=f"{inst.name}-waitsplit-{j}",
                            engine=inst.engine,
                            ins=[], outs=[],
                            sync_info=mybir.SyncInfo(
                                on_wait=overflow[j:j + MAX_WAITS], on_update=[]),
                        ))
                new_list.append(inst)
            if changed:
                bb.instructions[:] = new_list
    return nc


def _act_direct(nc, out, in_, func, bias=0.0, scale=1.0):
    """emit InstActivation directly (bass blocks Reciprocal/Rsqrt)."""
    ins = [nc.scalar.lower_ap(in_)]
    for v in (bias, scale, 0.0):
        if isinstance(v, (int, float)):
            ins.append(mybir.ImmediateValue(dtype=mybir.dt.float32, value=float(v)))
        else:
            ins.append(nc.scalar.lower_ap(v))
    return nc.scalar.add_instruction(
        mybir.InstActivation(
            name=nc.get_next_instruction_name(),
            func=func, ins=ins, outs=[nc.scalar.lower_ap(out)]))


def _build_nc():
    nc = bass.Bass()

    xt_d = nc.dram_tensor("xt", [D, T], BF16, kind="ExternalInput")
    # fp8 x for the f/i gate matmuls (DoubleRow): [c, p, i, t], ch = c*256+i*128+p
    xf8_d = nc.dram_tensor("xf8", [2, 128, 2, T], FP8, kind="ExternalInput")
    # fp8 f/i weights (x WS), [layer*2+gate, p, c, i, dout]
    wfi_d = nc.dram_tensor("wfi", [4, 128, 2, 2, D], FP8, kind="ExternalInput")
    # bf16 h-gate weights only: [layer, din, dout]
    wt_d = nc.dram_tensor("wt", [2, D, D], BF16, kind="ExternalInput")
    # f/i gate biases (layer-2 ones pre-corrected): [128, layer, gate*4+g]
    bias_d = nc.dram_tensor("bias", [128, 2, 8], F32, kind="ExternalInput")
    # scan initial state columns (-bh_eff): [128, layer, g]
    gi_d = nc.dram_tensor("gi", [128, 2, G], F32, kind="ExternalInput")
    # stats lhsT per (g,t): col t = 1 (s1), col 32+t = wg, col 64+t = c
    slt_d = nc.dram_tensor("slt", [G, NT, 128, 72], BF16, kind="ExternalInput")
    epi_d = nc.dram_tensor("epi", [NT, 8], F32, kind="ExternalInput")
    out_d = nc.dram_tensor("out", [NT, TT], F32, kind="ExternalOutput")

    with tile.TileContext(nc) as tc:
        with (
            tc.tile_pool(name="const", bufs=1) as const,
            tc.tile_pool(name="xtp", bufs=1) as xtp,
            tc.tile_pool(name="gp", bufs=1) as gp,        # wide per-g phase bufs
            tc.tile_pool(name="work", bufs=2) as work,    # i tiles
            tc.tile_pool(name="wk2", bufs=2) as wk2,      # res/sq/bgate/carry
            tc.tile_pool(name="fin", bufs=4) as fin,
            tc.tile_pool(name="gates_ps", bufs=3, space="PSUM") as gates_ps,
            tc.tile_pool(name="stats_ps", bufs=1, space="PSUM") as stats_ps,
        ):
            # ---- constants + x, DMA-ordered so phase (0,0) starts ASAP ----
            wt_sb = [None] * 2
            wfi_sb = [None] * 4
            def _load_wh(idx):
                w = const.tile([128, K, D], BF16, tag=f"wt{idx}", name=f"wt{idx}")
                nc.sync.dma_start(
                    out=w[:], in_=wt_d[idx].rearrange("(k p) d -> p k d", p=128))
                wt_sb[idx] = w
            def _load_wfi(idx, eng=None):
                w = const.tile([128, 2, 2, D], FP8, tag=f"wfi{idx}", name=f"wfi{idx}")
                (eng or nc.sync).dma_start(out=w[:], in_=wfi_d[idx])
                wfi_sb[idx] = w
            bias_sb = const.tile([128, 2, 8], F32)
            nc.sync.dma_start(out=bias_sb[:], in_=bias_d[:])
            _load_wfi(0)
            _load_wfi(1)
            xf8_sb = []
            W2F = 2 * TT
            for c in range(2):
                xc = xtp.tile([128, 2, T], FP8, tag=f"xf8{c}", name=f"xf8{c}")
                nc.gpsimd.dma_start(out=xc[:, :, 0:W2F], in_=xf8_d[c, :, :, 0:W2F])
                xf8_sb.append(xc)
            for c in range(2):
                nc.gpsimd.dma_start(
                    out=xf8_sb[c][:, :, W2F:HALF], in_=xf8_d[c, :, :, W2F:HALF])
            for c in range(2):
                nc.gpsimd.dma_start(
                    out=xf8_sb[c][:, :, HALF:T], in_=xf8_d[c, :, :, HALF:T])
            xt_sb = []
            for g in range(G):
                xx = xtp.tile([128, T], BF16, tag=f"xt{g}", name=f"xt{g}")
                xt_sb.append(xx)
            for h in range(NH):
                for g in range(G):
                    nc.gpsimd.dma_start(
                        out=xt_sb[g][:, h * HALF:(h + 1) * HALF],
                        in_=xt_d[g * 128:(g + 1) * 128, h * HALF:(h + 1) * HALF])
            _load_wh(0)
            gi_sb = const.tile([128, 2, G], F32)
            nc.sync.dma_start(out=gi_sb[:], in_=gi_d[:])
            _load_wfi(2)
            _load_wfi(3)
            _load_wh(1)
            slt_sb = const.tile([128, G, NT, 72], BF16)
            nc.sync.dma_start(
                out=slt_sb[:], in_=slt_d.rearrange("g t p c -> p g t c"))
            epi_sb = const.tile([NT, 8], F32)
            nc.sync.dma_start(out=epi_sb[:], in_=epi_d[:])
            # fp8 copies of g1 for the layer-2 f/i matmuls; alias the xf8
            # slots (xf8 is dead after the last layer-0 f/i matmul).
            gf8_sb = [None, None]

            # layer-1 scan outputs (bf16), resident per (g, half)
            g1_sb = [[None] * NH for _ in range(G)]
            # persistent stats accumulators (PSUM)
            s13_ps = stats_ps.tile([72, TT], F32, tag="s13")
            s2_ps = stats_ps.tile([NT, TT], F32, tag="s2")
            stats_first = [True]
            stats_count = [0]
            N_STATS = G * NT         # stats matmul pairs = 32

            def sig_phase(layer, t0, nt, gate_r):
                """pf/pi matmuls + sigmoids + den for tiles [t0, t0+nt).
                gate_r: previous phase's reciprocal tiles (or None), min-
                gated into the biases (forces ScalarE phase ordering)."""
                if gate_r is None:
                    bsrc = lambda gate, g: bias_sb[:, layer, 4 * gate + g:4 * gate + g + 1]
                else:
                    bg = wk2.tile([128, 8], F32, tag="bgate")
                    nc.vector.tensor_tensor(
                        bg[:], bias_sb[:, layer], gate_r[G - 1][:, 0:8], OP.min)
                    bsrc = lambda gate, g: bg[:, 4 * gate + g:4 * gate + g + 1]
                f_t, den_t = [], []
                for g in range(G):
                    f_t.append(gp.tile([128, nt * TT], BF16, tag=f"fr{g}", bufs=2, name=f"f{g}"))
                    den_t.append(gp.tile([128, nt * TT], BF16, tag=f"da{g}", bufs=2, name=f"den{g}"))
                src8 = xf8_sb if layer == 0 else gf8_sb
                W2 = 2 * TT
                for tp in range(nt // 2):
                    for g in range(G):
                        pf = gates_ps.tile([128, W2], F32, tag="mm", name="pf")
                        pi = gates_ps.tile([128, W2], F32, tag="mm", name="pi")
                        for gate, ps in ((0, pf), (1, pi)):
                            w8 = wfi_sb[2 * layer + gate]
                            for sub in range(2):
                                t = t0 + tp * 2 + sub
                                for c in range(2):
                                    nc.tensor.matmul(
                                        ps[:, sub * TT:(sub + 1) * TT],
                                        w8[:, c, :, g * 128:(g + 1) * 128],
                                        src8[c][:, :, t * TT:(t + 1) * TT],
                                        start=(c == 0), stop=(c == 1), perf_mode=DR)
                        fs = f_t[g][:, tp * W2:(tp + 1) * W2]
                        nc.scalar.activation(fs, pf[:], AF.Sigmoid, bias=bsrc(0, g),
                                             scale=1.0 / WS)
                        i_sb = work.tile([128, W2], BF16, tag="i")
                        nc.scalar.activation(i_sb[:], pi[:], AF.Sigmoid, bias=bsrc(1, g),
                                             scale=1.0 / WS)
                        nc.vector.tensor_add(
                            den_t[g][:, tp * W2:(tp + 1) * W2], fs, i_sb[:])
                        i_last = i_sb
                # gate all reciprocals on the last sigmoid of the phase:
                # max(den, i) == den exactly (den = f+i >= i), so this only
                # adds the dependency, keeping the act-table phases contiguous.
                for g in range(G):
                    nc.vector.tensor_tensor(
                        den_t[g][:, 0:1], den_t[g][:, 0:1], i_last[:, 0:1], OP.max)
                return f_t, den_t

            def rec_phase(layer, t0, nt, f_t, den_t, g2_carry):
                """reciprocal + a + u' for tiles [t0, t0+nt); scans are
                emitted later (emit_scans closure) so the next sig phase's
                den adds aren't queued behind them on DVE."""
                r_t, a_t, up_t = [], [], []
                for g in range(G):
                    r = gp.tile([128, nt * TT], BF16, tag=f"fr{g}", bufs=2, name=f"r{g}")
                    _act_direct(nc, r[:], den_t[g][:], AF.Reciprocal)
                    r_t.append(r)
                for g in range(G):
                    a = gp.tile([128, nt * TT], BF16, tag=f"da{g}", bufs=2, name=f"a{g}")
                    nc.vector.tensor_mul(a[:], f_t[g][:], r_t[g][:])
                    a_t.append(a)
                    up_t.append(gp.tile([128, nt * TT], BF16, tag=f"up{g}", bufs=1, name=f"up{g}"))
                W2 = 2 * TT
                def zh_pair(tp):
                    for g in range(G):
                        ph = gates_ps.tile([128, W2], F32, tag="mm", name="ph")
                        w = wt_sb[layer]
                        for sub in range(2):
                            t = t0 + tp * 2 + sub
                            for k in range(K):
                                if layer == 0:
                                    r = xt_sb[k][:, t * TT:(t + 1) * TT]
                                else:
                                    r = g1_sb[k][t // TPH][:, (t % TPH) * TT:(t % TPH + 1) * TT]
                                nc.tensor.matmul(
                                    ph[:, sub * TT:(sub + 1) * TT],
                                    w[:, k, g * 128:(g + 1) * 128], r,
                                    start=(k == 0), stop=(k == K - 1))
                        nc.vector.scalar_tensor_tensor(
                            up_t[g][:, tp * W2:(tp + 1) * W2],
                            a_t[g][:, tp * W2:(tp + 1) * W2], 1.0, ph[:],
                            OP.subtract, OP.mult)
                for tp in range(nt // 2):
                    zh_pair(tp)
                def emit_zh_last():
                    pass
                cast_list = []

                def emit_casts():
                    for dst, src_ap in cast_list:
                        nc.scalar.activation(dst, src_ap, AF.Copy)

                def emit_scans(epi_g=None):
                    gout = []
                    carry = [] if (layer == 1 and t0 + nt < NT) else None
                    for g in range(G):
                        if layer == 0:
                            half = t0 // TPH
                            go = gp.tile([128, nt * TT], BF16, tag=f"g1_{g}_{half}", name=f"g1_{g}_{half}")
                            init = (gi_sb[:, 0, g:g + 1] if t0 == 0
                                    else g1_sb[g][0][:, HALF - 1:HALF])
                            g1_sb[g][half] = go
                        else:
                            go = gp.tile([128, nt * TT], BF16, tag=f"g1_{g}_0", name=f"g2_{g}_{t0}")
                            init = (gi_sb[:, 1, g:g + 1] if t0 == 0
                                    else g2_carry[g][:])
                        nc.vector.tensor_tensor_scan(
                            go[:], a_t[g][:], up_t[g][:], init, OP.mult, OP.subtract)
                        gout.append(go)
                        if layer == 0:
                            c8, i8 = g // 2, g % 2
                            if gf8_sb[c8] is None:
                                gf8_sb[c8] = xtp.tile(
                                    [128, 2, T], FP8, tag=f"xf8{c8}", name=f"gf8{c8}")
                            nc.scalar.activation(
                                gf8_sb[c8][:, i8, t0 * TT:(t0 + nt) * TT],
                                go[:], AF.Copy)
                        if carry is not None:
                            cr = wk2.tile([128, 1], BF16, tag=f"carry{g}", name=f"carry{g}")
                            nc.vector.tensor_copy(cr[:], go[:, nt * TT - 1:nt * TT])
                            carry.append(cr)
                        if epi_g is not None:
                            epi_g(g, go)
                    return gout, carry
                return r_t, emit_scans, emit_casts

            def epilogue(t0, nt):
                """per-g closure: res = g2 + x^T, square, stats matmuls."""
                def epi_g(g, g2):
                    res = wk2.tile([128, nt * TT], BF16, tag="res", bufs=1)
                    nc.vector.tensor_add(
                        res[:], g2[:],
                        xt_sb[g][:, t0 * TT:(t0 + nt) * TT])
                    sq = wk2.tile([128, nt * TT], BF16, tag="sq", bufs=1)
                    nc.scalar.activation(sq[:], res[:], AF.Square)
                    for ti in range(nt):
                        t = t0 + ti
                        rs = res[:, ti * TT:(ti + 1) * TT]
                        sqs = sq[:, ti * TT:(ti + 1) * TT]
                        first = stats_first[0]
                        stats_first[0] = False
                        stats_count[0] += 1
                        last = stats_count[0] == N_STATS
                        nc.tensor.matmul(
                            s13_ps[:], slt_sb[:, g, t, 0:72], rs,
                            start=first, stop=last, skip_group_check=True)
                        nc.tensor.matmul(
                            s2_ps[:], slt_sb[:, g, t, 0:8], sqs,
                            start=first, stop=last, skip_group_check=True)
                return epi_g

            # ---- pipeline (scans deferred past the next sig phase);
            #      layer-1 split [0-3],[4-5],[6-7] to shrink the tail ----
            f_t, den_t = sig_phase(0, 0, 4, None)
            r_a, sc_a, cast_a = rec_phase(0, 0, 4, f_t, den_t, None)
            f_t, den_t = sig_phase(0, 4, 4, r_a)
            sc_a()
            r_b, sc_b, cast_b = rec_phase(0, 4, 4, f_t, den_t, None)
            cast_a()
            f_t, den_t = sig_phase(1, 0, 4, r_b)
            sc_b()
            r_c, sc_c, _ = rec_phase(1, 0, 4, f_t, den_t, None)
            cast_b()
            f_t, den_t = sig_phase(1, 4, 2, r_c)
            _, carry1 = sc_c(epilogue(0, 4))
            r_d, sc_d, _ = rec_phase(1, 4, 2, f_t, den_t, carry1)
            f_t, den_t = sig_phase(1, 6, 2, r_d)
            _, carry2 = sc_d(epilogue(4, 2))
            r_e, sc_e, _ = rec_phase(1, 6, 2, f_t, den_t, carry2)
            # preload the rsqrt act-table during the last scan window
            # (gated on the final reciprocal so it can't reorder earlier)
            rsq_pre = wk2.tile([NT, 1], F32, tag="rsqpre")
            _act_direct(nc, rsq_pre[:], r_e[G - 1][0:NT, 0:1], AF.Rsqrt)
            sc_e(epilogue(6, 2))

            # ---- final LN + projection math on [8, 512] ----
            # y = -( (s1_0*A - s3_0) + Kc ) * rsqrt(v + eps') + c0
            # v  = (s2_0 + 2*sc)/D - ((s1_0 + C1)/D)^2
            sc_sb = fin.tile([NT, TT], F32, tag="fin")
            nc.scalar.activation(sc_sb[:], s13_ps[64:64 + NT, :], AF.Copy)
            s3_sb = fin.tile([NT, TT], F32, tag="fin")
            nc.scalar.activation(s3_sb[:], s13_ps[32:32 + NT, :], AF.Copy)
            s2c = fin.tile([NT, TT], F32, tag="fin")
            nc.vector.scalar_tensor_tensor(
                s2c[:], sc_sb[:], 2.0, s2_ps[:], OP.mult, OP.add)
            mu2 = fin.tile([NT, TT], F32, tag="fin")
            nc.scalar.activation(mu2[:], s13_ps[0:NT, :], AF.Square,
                                 bias=epi_sb[:, 3:4], scale=1.0 / D)
            v = fin.tile([NT, TT], F32, tag="fin")
            nc.vector.scalar_tensor_tensor(
                v[:], s2c[:], 1.0 / D, mu2[:], OP.mult, OP.subtract)
            rv = fin.tile([NT, TT], F32, tag="fin")
            _act_direct(nc, rv[:], v[:], AF.Rsqrt, bias=epi_sb[:, 2:3])
            q = fin.tile([NT, TT], F32, tag="fin")
            nc.vector.scalar_tensor_tensor(
                q[:], s13_ps[0:NT, :], epi_sb[:, 1:2], s3_sb[:],
                OP.mult, OP.subtract)
            z = fin.tile([NT, TT], F32, tag="fin")
            nc.vector.scalar_tensor_tensor(
                z[:], q[:], epi_sb[:, 4:5], rv[:], OP.add, OP.mult)
            o_sb = fin.tile([NT, TT], F32, tag="fin")
            nc.scalar.activation(o_sb[:], z[:], AF.Identity,
                                 bias=epi_sb[:, 0:1], scale=-1.0)
            nc.sync.dma_start(out=out_d[:], in_=o_sb[:])

    _split_excess_waits(nc)
    return nc


_NC_CACHE = None


def _get_nc():
    global _NC_CACHE
    if _NC_CACHE is None:
        _NC_CACHE = _build_nc()
    return _NC_CACHE


def _host_prep(inputs):
    x = np.asarray(inputs["x"], dtype=np.float32)
    Ws = [np.asarray(inputs[n], np.float32) for n in
          ("Wf0", "Wi0", "Wh0", "Wf1", "Wi1", "Wh1")]
    bs = [np.asarray(inputs[n], np.float32) for n in
          ("bf0", "bi0", "bh0", "bf1", "bi1", "bh1")]
    bf0, bi0, bh0, bf1, bi1, bh1 = bs
    Wf1, Wi1, Wh1 = Ws[3], Ws[4], Ws[5]
    # h-bias elimination: layer-2 gate biases absorb Wx1 @ bh0
    bf1e = bf1 + Wf1 @ bh0
    bi1e = bi1 + Wi1 @ bh0
    bh1e = bh1 + Wh1 @ bh0

    # bf16 h-gate weights only
    wt_all = np.ascontiguousarray(
        np.stack([Ws[2].T, Ws[5].T])).astype(ml_dtypes.bfloat16)  # [2, din, dout]
    # fp8 f/i weights, scaled by WS (undone via the sigmoid input scale) to
    # keep them out of the fp8 subnormal range; [l*2+gate, p, c, i, dout]
    np_fp8 = mybir.dt.np(mybir.dt.float8e4)
    wfi = np.stack([
        (Ws[j].T * WS).reshape(2, 2, 128, D).transpose(2, 0, 1, 3)
        for j in (0, 1, 3, 4)
    ]).astype(np_fp8)

    bias = np.zeros((128, 2, 8), np.float32)
    gi = np.zeros((128, 2, G), np.float32)
    for g in range(G):
        sl = slice(g * 128, (g + 1) * 128)
        bias[:, 0, 0 * 4 + g] = bf0[sl]
        bias[:, 0, 1 * 4 + g] = bi0[sl]
        bias[:, 1, 0 * 4 + g] = bf1e[sl]
        bias[:, 1, 1 * 4 + g] = bi1e[sl]
        gi[:, 0, g] = -bh0[sl]
        gi[:, 1, g] = -bh1e[sl]
    # min-gate trick requires |bias| < 0.5 <= r = 1/(f+i)
    assert np.abs(bias).max() < 0.45, "bias magnitude breaks min-gate trick"

    w_out = np.asarray(inputs["W_out"], np.float32).reshape(D)
    ln_g = np.asarray(inputs["ln_g"], np.float32)
    ln_b = np.asarray(inputs["ln_b"], np.float32)
    b_out = float(np.asarray(inputs["b_out"], np.float32).reshape(()))
    wg = w_out * ln_g
    c = bh1e                         # constant channel shift of res
    c0 = float(w_out @ ln_b) + b_out
    swg = float(wg.sum())
    C1 = float(c.sum())
    C2 = float((c * c).sum())
    C3 = float((wg * c).sum())
    A = swg / D
    Kc = C1 * A - C3
    epsP = LN_EPS + C2 / D
    C1D = C1 / D

    slt = np.zeros((G, NT, 128, 72), np.float32)
    for g in range(G):
        sl = slice(g * 128, (g + 1) * 128)
        for t in range(NT):
            slt[g, t, :, t] = 1.0
            slt[g, t, :, 32 + t] = wg[sl]
            slt[g, t, :, 64 + t] = c[sl]
    slt = slt.astype(ml_dtypes.bfloat16)

    epi = np.zeros((NT, 8), np.float32)
    epi[:, 0] = c0
    epi[:, 1] = A
    epi[:, 2] = epsP
    epi[:, 3] = C1D
    epi[:, 4] = Kc

    xt = x.transpose(0, 2, 1)                              # [B, D, T]
    xt_b = np.ascontiguousarray(xt).astype(ml_dtypes.bfloat16)
    # fp8 x chunks for DoubleRow: [B, c, p, i, t], channel = c*256+i*128+p
    xf8 = np.ascontiguousarray(
        xt.reshape(B, 2, 2, 128, T).transpose(0, 1, 3, 2, 4)).astype(np_fp8)
    return xt_b, xf8, wt_all, wfi, bias, gi, slt, epi


def _in_maps(inputs):
    xt_b, xf8, wt_all, wfi, bias, gi, slt, epi = _host_prep(inputs)
    return [
        {
            "xt": xt_b[b], "xf8": xf8[b],
            "wt": wt_all, "wfi": wfi, "bias": bias, "gi": gi,
            "slt": slt, "epi": epi,
        }
        for b in range(B)
    ]


def kernel(**inputs):
    nc = _get_nc()
    res = run_bass_kernel_spmd(nc, _in_maps(inputs), list(range(B)))
    out = np.stack([res.results[b]["out"].reshape(T, OUT) for b in range(B)])
    return out.astype(np.float32)


def kernel_traced(**inputs):
    """same as kernel() but returns (output, BassKernelResults) with timing"""
    nc = _get_nc()
    res = run_bass_kernel_spmd(nc, _in_maps(inputs), list(range(B)), trace=True)
    out = np.stack([res.results[b]["out"].reshape(T, OUT) for b in range(B)])
    return out.astype(np.float32), res


# revision 41
# speedup vs baseline: 1.0074x; 1.0074x over previous
"""Trainium2 Bass kernel for nn_DecoderMinLSTMGNN.

Model (per sample): two MinLSTM layers (D=512) over T=4096 steps, residual,
LayerNorm, projection D->1.  B=8 samples are data-parallel across the 8
NeuronCores (one sample per core).  344.5us baseline -> ~240us.

Layout is channels-major: x^T [D, T].  The time-dim linear recurrence
h_t = a_t*h_{t-1} + (1-a_t)*htilde_t runs on the VectorE TensorTensorScan
(one independent recurrence per partition, scanned along the free dim).

Key optimizations:
- h-gate bias elimination (g = h - bh substitution): the recurrence becomes
  bias-free (init -bh); the bias folds into the next layer's f/i gate
  biases (bf1_eff = bf1 + Wf1 @ bh0) and into the LN/projection stats via
  an extra lhsT column (c = bh1_eff) + host-side constants.  Removes all
  64 bias matmuls.
- fp8 (e4m3) DoubleRow matmuls for the f/i gate projections (half the
  cycles/row of bf16): weights are scaled by 64 into fp8's normal range
  and the scale is undone for free via the sigmoid's input-scale field.
  Gate noise is strongly damped (sigmoid slope, a in (0,1), contractive
  scan), so fp8 costs <1e-3 of final rel-err.  The h-gate (htilde path)
  and the LN stats stay bf16; PSUM and scan state are fp32.
- ScalarE act-table phase batching: sigmoid and reciprocal live in
  different act-table sets (1.3us per reload; naive interleave costs ~97
  loads).  Work is organized in (layer, half-of-T) phases: 32 sigmoids,
  then 4 wide reciprocals.  Two artificial-dependency tricks keep the
  scheduler from interleaving phases, both numerically exact:
    min-gate:  next phase's biases pass through min(bias, r) with
               |bias| < 0.45 <= 0.5 <= r = 1/(f+i);
    max-gate:  den[:,0:1] = max(den[:,0:1], i_last) since den = f+i >= i.
  Result: 9 table loads total.
- Deferred scans: each phase's 4 wide scans (the serial DVE hub) are
  emitted after the NEXT sig phase so the priority-heap scheduler uses
  them as gap filler instead of queueing dens behind them.
- bf16 everywhere else (DVE 2x tensor-tensor mode, half SBUF/DMA) and
  wide [128,1024/2048] instructions to amortize per-instruction overhead.
- SBUF slot aliasing (f/r share a ring, den/a share a ring, g2 reuses the
  g1 half-0 slots, gf8 reuses the xf8 slots) to fit in 208KB/partition.
- Epilogue: res/square/stats matmuls interleaved per channel-group with
  the layer-2 scans; LN + projection collapse into 3 matmul-accumulated
  row-stats (s1, s3 = wg.res, sc = c.res, s2 = res^2) + a short fp32
  epilogue on [8,512] tiles.
"""

import numpy as np
import ml_dtypes

import concourse.bass as bass
import concourse.mybir as mybir
import concourse.tile as tile
from concourse.bass_utils import run_bass_kernel_spmd

F32 = mybir.dt.float32
BF16 = mybir.dt.bfloat16
FP8 = mybir.dt.float8e4
DR = mybir.MatmulPerfMode.DoubleRow
WS = 64.0
AF = mybir.ActivationFunctionType
OP = mybir.AluOpType

B, T, D = 8, 4096, 512
OUT = 1
LN_EPS = 1e-5
TT = 512                 # time-tile size
NT = T // TT             # 8 time tiles
G = D // 128             # 4 channel groups
K = D // 128             # 4 contraction chunks
TPH = 4                  # time tiles per phase (half)
HALF = TPH * TT          # 2048
NH = NT // TPH           # 2 halves

MAX_WAITS = 1


def _split_excess_waits(nc):
    """walrus in this container rejects >1 semaphore wait per instruction
    ("Too many sync wait commands"); move excess waits onto NoOps."""
    for fn in nc.m.functions:
        for bb in fn.blocks:
            new_list = []
            changed = False
            for inst in bb.instructions:
                si = inst.sync_info
                waits = list(si.on_wait) if si is not None and si.on_wait else []
                if len(waits) > MAX_WAITS:
                    changed = True
                    overflow = waits[:-MAX_WAITS]
                    si.on_wait = waits[-MAX_WAITS:]
                    for j in range(0, len(overflow), MAX_WAITS):
                        new_list.append(mybir.InstNoOp(
                            name=f"{inst.name}-waitsplit-{j}",
                            engine=inst.engine,
                            ins=[], outs=[],
                            sync_info=mybir.SyncInfo(
                                on_wait=overflow[j:j + MAX_WAITS], on_update=[]),
                        ))
                new_list.append(inst)
            if changed:
                bb.instructions[:] = new_list
    return nc


def _act_direct(nc, out, in_, func, bias=0.0, scale=1.0):
    """emit InstActivation directly (bass blocks Reciprocal/Rsqrt)."""
    ins = [nc.scalar.lower_ap(in_)]
    for v in (bias, scale, 0.0):
        if isinstance(v, (int, float)):
            ins.append(mybir.ImmediateValue(dtype=mybir.dt.float32, value=float(v)))
        else:
            ins.append(nc.scalar.lower_ap(v))
    return nc.scalar.add_instruction(
        mybir.InstActivation(
            name=nc.get_next_instruction_name(),
            func=func, ins=ins, outs=[nc.scalar.lower_ap(out)]))


def _build_nc():
    nc = bass.Bass()

    xt_d = nc.dram_tensor("xt", [D, T], BF16, kind="ExternalInput")
    # fp8 x for the f/i gate matmuls (DoubleRow): [c, p, i, t], ch = c*256+i*128+p
    xf8_d = nc.dram_tensor("xf8", [2, 128, 2, T], FP8, kind="ExternalInput")
    # fp8 f/i weights (x WS), [layer*2+gate, p, c, i, dout]
    wfi_d = nc.dram_tensor("wfi", [4, 128, 2, 2, D], FP8, kind="ExternalInput")
    # bf16 h-gate weights only: [layer, din, dout]
    wt_d = nc.dram_tensor("wt", [2, D, D], BF16, kind="ExternalInput")
    # f/i gate biases (layer-2 ones pre-corrected): [128, layer, gate*4+g]
    bias_d = nc.dram_tensor("bias", [128, 2, 8], F32, kind="ExternalInput")
    # scan initial state columns (-bh_eff): [128, layer, g]
    gi_d = nc.dram_tensor("gi", [128, 2, G], F32, kind="ExternalInput")
    # stats lhsT per (g,t): col t = 1 (s1), col 32+t = wg, col 64+t = c
    slt_d = nc.dram_tensor("slt", [G, NT, 128, 72], BF16, kind="ExternalInput")
    epi_d = nc.dram_tensor("epi", [NT, 8], F32, kind="ExternalInput")
    out_d = nc.dram_tensor("out", [NT, TT], F32, kind="ExternalOutput")

    with tile.TileContext(nc) as tc:
        with (
            tc.tile_pool(name="const", bufs=1) as const,
            tc.tile_pool(name="xtp", bufs=1) as xtp,
            tc.tile_pool(name="gp", bufs=1) as gp,        # wide per-g phase bufs
            tc.tile_pool(name="work", bufs=2) as work,    # i tiles
            tc.tile_pool(name="wk2", bufs=2) as wk2,      # res/sq/bgate/carry
            tc.tile_pool(name="fin", bufs=4) as fin,
            tc.tile_pool(name="gates_ps", bufs=3, space="PSUM") as gates_ps,
            tc.tile_pool(name="stats_ps", bufs=1, space="PSUM") as stats_ps,
        ):
            # ---- constants + x, DMA-ordered so phase (0,0) starts ASAP ----
            wt_sb = [None] * 2
            wfi_sb = [None] * 4
            def _load_wh(idx):
                w = const.tile([128, K, D], BF16, tag=f"wt{idx}", name=f"wt{idx}")
                nc.sync.dma_start(
                    out=w[:], in_=wt_d[idx].rearrange("(k p) d -> p k d", p=128))
                wt_sb[idx] = w
            def _load_wfi(idx):
                w = const.tile([128, 2, 2, D], FP8, tag=f"wfi{idx}", name=f"wfi{idx}")
                nc.sync.dma_start(out=w[:], in_=wfi_d[idx])
                wfi_sb[idx] = w
            bias_sb = const.tile([128, 2, 8], F32)
            nc.sync.dma_start(out=bias_sb[:], in_=bias_d[:])
            _load_wfi(0)
            _load_wfi(1)
            xf8_sb = []
            W2F = 2 * TT
            for c in range(2):
                xc = xtp.tile([128, 2, T], FP8, tag=f"xf8{c}", name=f"xf8{c}")
                nc.gpsimd.dma_start(out=xc[:, :, 0:W2F], in_=xf8_d[c, :, :, 0:W2F])
                xf8_sb.append(xc)
            for c in range(2):
                nc.gpsimd.dma_start(
                    out=xf8_sb[c][:, :, W2F:HALF], in_=xf8_d[c, :, :, W2F:HALF])
            for c in range(2):
                nc.gpsimd.dma_start(
                    out=xf8_sb[c][:, :, HALF:T], in_=xf8_d[c, :, :, HALF:T])
            xt_sb = []
            for g in range(G):
                xx = xtp.tile([128, T], BF16, tag=f"xt{g}", name=f"xt{g}")
                xt_sb.append(xx)
            for h in range(NH):
                for g in range(G):
                    nc.gpsimd.dma_start(
                        out=xt_sb[g][:, h * HALF:(h + 1) * HALF],
                        in_=xt_d[g * 128:(g + 1) * 128, h * HALF:(h + 1) * HALF])
            _load_wh(0)
            gi_sb = const.tile([128, 2, G], F32)
            nc.sync.dma_start(out=gi_sb[:], in_=gi_d[:])
            _load_wfi(2)
            _load_wfi(3)
            _load_wh(1)
            slt_sb = const.tile([128, G, NT, 72], BF16)
            nc.sync.dma_start(
                out=slt_sb[:], in_=slt_d.rearrange("g t p c -> p g t c"))
            epi_sb = const.tile([NT, 8], F32)
            nc.sync.dma_start(out=epi_sb[:], in_=epi_d[:])
            # fp8 copies of g1 for the layer-2 f/i matmuls; alias the xf8
            # slots (xf8 is dead after the last layer-0 f/i matmul).
            gf8_sb = [None, None]

            # layer-1 scan outputs (bf16), resident per (g, half)
            g1_sb = [[None] * NH for _ in range(G)]
            # persistent stats accumulators (PSUM)
            s13_ps = stats_ps.tile([72, TT], F32, tag="s13")
            s2_ps = stats_ps.tile([NT, TT], F32, tag="s2")
            stats_first = [True]
            stats_count = [0]
            N_STATS = G * NT         # stats matmul pairs = 32

            def sig_phase(layer, t0, nt, gate_r):
                """pf/pi matmuls + sigmoids + den for tiles [t0, t0+nt).
                gate_r: previous phase's reciprocal tiles (or None), min-
                gated into the biases (forces ScalarE phase ordering)."""
                if gate_r is None:
                    bsrc = lambda gate, g: bias_sb[:, layer, 4 * gate + g:4 * gate + g + 1]
                else:
                    # high priority: this tiny DVE op gates the whole next
                    # sigmoid block; without it the scheduler queues it
                    # behind ~6us of up-STTs.
                    with tc.high_priority():
                        bg = wk2.tile([128, 8], F32, tag="bgate")
                        nc.vector.tensor_tensor(
                            bg[:], bias_sb[:, layer], gate_r[0][:, 0:8], OP.min)
                        bg2 = wk2.tile([128, 8], F32, tag="bgate", name="bg2")
                        nc.vector.tensor_tensor(
                            bg2[:], bg[:], gate_r[G - 1][:, 0:8], OP.min)
                        bg = bg2
                    bsrc = lambda gate, g: bg[:, 4 * gate + g:4 * gate + g + 1]
                f_t, den_t = [], []
                for g in range(G):
                    f_t.append(gp.tile([128, nt * TT], BF16, tag=f"fr{g}", bufs=2, name=f"f{g}"))
                    den_t.append(gp.tile([128, nt * TT], BF16, tag=f"da{g}", bufs=2, name=f"den{g}"))
                src8 = xf8_sb if layer == 0 else gf8_sb
                W2 = 2 * TT
                for tp in range(nt // 2):
                    for g in range(G):
                        pf = gates_ps.tile([128, W2], F32, tag="mm", name="pf")
                        pi = gates_ps.tile([128, W2], F32, tag="mm", name="pi")
                        for gate, ps in ((0, pf), (1, pi)):
                            w8 = wfi_sb[2 * layer + gate]
                            for sub in range(2):
                                t = t0 + tp * 2 + sub
                                for c in range(2):
                                    nc.tensor.matmul(
                                        ps[:, sub * TT:(sub + 1) * TT],
                                        w8[:, c, :, g * 128:(g + 1) * 128],
                                        src8[c][:, :, t * TT:(t + 1) * TT],
                                        start=(c == 0), stop=(c == 1), perf_mode=DR)
                        fs = f_t[g][:, tp * W2:(tp + 1) * W2]
                        nc.scalar.activation(fs, pf[:], AF.Sigmoid, bias=bsrc(0, g),
                                             scale=1.0 / WS)
                        i_sb = work.tile([128, W2], BF16, tag="i")
                        nc.scalar.activation(i_sb[:], pi[:], AF.Sigmoid, bias=bsrc(1, g),
                                             scale=1.0 / WS)
                        nc.vector.tensor_add(
                            den_t[g][:, tp * W2:(tp + 1) * W2], fs, i_sb[:])
                        i_last = i_sb
                # gate all reciprocals on the last sigmoid of the phase:
                # max(den, i) == den exactly (den = f+i >= i), so this only
                # adds the dependency, keeping the act-table phases contiguous.
                for g in range(G):
                    nc.vector.tensor_tensor(
                        den_t[g][:, 0:1], den_t[g][:, 0:1], i_last[:, 0:1], OP.max)
                return f_t, den_t

            def rec_phase(layer, t0, nt, f_t, den_t, g2_carry):
                """reciprocal + a + u' for tiles [t0, t0+nt); scans are
                emitted later (emit_scans closure) so the next sig phase's
                den adds aren't queued behind them on DVE."""
                r_t, a_t, up_t = [], [], []
                for g in range(G):
                    r = gp.tile([128, nt * TT], BF16, tag=f"fr{g}", bufs=2, name=f"r{g}")
                    _act_direct(nc, r[:], den_t[g][:], AF.Reciprocal)
                    r_t.append(r)
                for g in range(G):
                    a = gp.tile([128, nt * TT], BF16, tag=f"da{g}", bufs=2, name=f"a{g}")
                    nc.vector.tensor_mul(a[:], f_t[g][:], r_t[g][:])
                    a_t.append(a)
                    up_t.append(gp.tile([128, nt * TT], BF16, tag=f"up{g}", bufs=1, name=f"up{g}"))
                W2 = 2 * TT
                def zh_pair(tp):
                    for g in range(G):
                        ph = gates_ps.tile([128, W2], F32, tag="mm", name="ph")
                        w = wt_sb[layer]
                        for sub in range(2):
                            t = t0 + tp * 2 + sub
                            for k in range(K):
                                if layer == 0:
                                    r = xt_sb[k][:, t * TT:(t + 1) * TT]
                                else:
                                    r = g1_sb[k][t // TPH][:, (t % TPH) * TT:(t % TPH + 1) * TT]
                                nc.tensor.matmul(
                                    ph[:, sub * TT:(sub + 1) * TT],
                                    w[:, k, g * 128:(g + 1) * 128], r,
                                    start=(k == 0), stop=(k == K - 1))
                        nc.vector.scalar_tensor_tensor(
                            up_t[g][:, tp * W2:(tp + 1) * W2],
                            a_t[g][:, tp * W2:(tp + 1) * W2], 1.0, ph[:],
                            OP.subtract, OP.mult)
                for tp in range(nt // 2):
                    zh_pair(tp)
                def emit_zh_last():
                    pass
                def emit_scans(epi_g=None):
                    gout = []
                    carry = [] if (layer == 1 and t0 + nt < NT) else None
                    for g in range(G):
                        if layer == 0:
                            half = t0 // TPH
                            go = gp.tile([128, nt * TT], BF16, tag=f"g1_{g}_{half}", name=f"g1_{g}_{half}")
                            init = (gi_sb[:, 0, g:g + 1] if t0 == 0
                                    else g1_sb[g][0][:, HALF - 1:HALF])
                            g1_sb[g][half] = go
                        else:
                            go = gp.tile([128, nt * TT], BF16, tag=f"g1_{g}_0", name=f"g2_{g}_{t0}")
                            init = (gi_sb[:, 1, g:g + 1] if t0 == 0
                                    else g2_carry[g][:])
                        nc.vector.tensor_tensor_scan(
                            go[:], a_t[g][:], up_t[g][:], init, OP.mult, OP.subtract)
                        gout.append(go)
                        if layer == 0:
                            c8, i8 = g // 2, g % 2
                            if gf8_sb[c8] is None:
                                gf8_sb[c8] = xtp.tile(
                                    [128, 2, T], FP8, tag=f"xf8{c8}", name=f"gf8{c8}")
                            nc.scalar.activation(
                                gf8_sb[c8][:, i8, t0 * TT:(t0 + nt) * TT],
                                go[:], AF.Copy)
                        if carry is not None:
                            cr = wk2.tile([128, 1], BF16, tag=f"carry{g}", name=f"carry{g}")
                            nc.vector.tensor_copy(cr[:], go[:, nt * TT - 1:nt * TT])
                            carry.append(cr)
                        if epi_g is not None:
                            epi_g(g, go)
                    return gout, carry
                return r_t, emit_scans, emit_zh_last

            def epilogue(t0, nt):
                """per-g closure: res = g2 + x^T, square, stats matmuls."""
                def epi_g(g, g2):
                    res = wk2.tile([128, nt * TT], BF16, tag="res", bufs=1)
                    nc.vector.tensor_add(
                        res[:], g2[:],
                        xt_sb[g][:, t0 * TT:(t0 + nt) * TT])
                    sq = wk2.tile([128, nt * TT], BF16, tag="sq", bufs=1)
                    nc.scalar.activation(sq[:], res[:], AF.Square)
                    for ti in range(nt):
                        t = t0 + ti
                        rs = res[:, ti * TT:(ti + 1) * TT]
                        sqs = sq[:, ti * TT:(ti + 1) * TT]
                        first = stats_first[0]
                        stats_first[0] = False
                        stats_count[0] += 1
                        last = stats_count[0] == N_STATS
                        nc.tensor.matmul(
                            s13_ps[:], slt_sb[:, g, t, 0:72], rs,
                            start=first, stop=last, skip_group_check=True)
                        nc.tensor.matmul(
                            s2_ps[:], slt_sb[:, g, t, 0:8], sqs,
                            start=first, stop=last, skip_group_check=True)
                return epi_g

            # ---- pipeline (scans deferred past the next sig phase);
            #      layer-1 split [0-3],[4-5],[6-7] to shrink the tail ----
            f_t, den_t = sig_phase(0, 0, 4, None)
            r_a, sc_a, zh_a = rec_phase(0, 0, 4, f_t, den_t, None)
            f_t, den_t = sig_phase(0, 4, 4, r_a)
            zh_a()
            sc_a()
            r_b, sc_b, zh_b = rec_phase(0, 4, 4, f_t, den_t, None)
            f_t, den_t = sig_phase(1, 0, 4, r_b)
            zh_b()
            sc_b()
            r_c, sc_c, zh_c = rec_phase(1, 0, 4, f_t, den_t, None)
            f_t, den_t = sig_phase(1, 4, 2, r_c)
            zh_c()
            _, carry1 = sc_c(epilogue(0, 4))
            r_d, sc_d, zh_d = rec_phase(1, 4, 2, f_t, den_t, carry1)
            f_t, den_t = sig_phase(1, 6, 2, r_d)
            zh_d()
            _, carry2 = sc_d(epilogue(4, 2))
            r_e, sc_e, zh_e = rec_phase(1, 6, 2, f_t, den_t, carry2)
            zh_e()
            # preload the rsqrt act-table during the last scan window
            # (gated on the final reciprocal so it can't reorder earlier)
            rsq_pre = wk2.tile([NT, 1], F32, tag="rsqpre")
            _act_direct(nc, rsq_pre[:], r_e[G - 1][0:NT, 0:1], AF.Rsqrt)
            sc_e(epilogue(6, 2))

            # ---- final LN + projection math on [8, 512] ----
            # y = -( (s1_0*A - s3_0) + Kc ) * rsqrt(v + eps') + c0
            # v  = (s2_0 + 2*sc)/D - ((s1_0 + C1)/D)^2
            sc_sb = fin.tile([NT, TT], F32, tag="fin")
            nc.scalar.activation(sc_sb[:], s13_ps[64:64 + NT, :], AF.Copy)
            s3_sb = fin.tile([NT, TT], F32, tag="fin")
            nc.scalar.activation(s3_sb[:], s13_ps[32:32 + NT, :], AF.Copy)
            s2c = fin.tile([NT, TT], F32, tag="fin")
            nc.vector.scalar_tensor_tensor(
                s2c[:], sc_sb[:], 2.0, s2_ps[:], OP.mult, OP.add)
            mu2 = fin.tile([NT, TT], F32, tag="fin")
            nc.scalar.activation(mu2[:], s13_ps[0:NT, :], AF.Square,
                                 bias=epi_sb[:, 3:4], scale=1.0 / D)
            v = fin.tile([NT, TT], F32, tag="fin")
            nc.vector.scalar_tensor_tensor(
                v[:], s2c[:], 1.0 / D, mu2[:], OP.mult, OP.subtract)
            rv = fin.tile([NT, TT], F32, tag="fin")
            _act_direct(nc, rv[:], v[:], AF.Rsqrt, bias=epi_sb[:, 2:3])
            q = fin.tile([NT, TT], F32, tag="fin")
            nc.vector.scalar_tensor_tensor(
                q[:], s13_ps[0:NT, :], epi_sb[:, 1:2], s3_sb[:],
                OP.mult, OP.subtract)
            z = fin.tile([NT, TT], F32, tag="fin")
            nc.vector.scalar_tensor_tensor(
                z[:], q[:], epi_sb[:, 4:5], rv[:], OP.add, OP.mult)
            o_sb = fin.tile([NT, TT], F32, tag="fin")
            nc.scalar.activation(o_sb[:], z[:], AF.Identity,
                                 bias=epi_sb[:, 0:1], scale=-1.0)
            nc.sync.dma_start(out=out_d[:], in_=o_sb[:])

    _split_excess_waits(nc)
    return nc


_NC_CACHE = None


def _get_nc():
    global _NC_CACHE
    if _NC_CACHE is None:
        _NC_CACHE = _build_nc()
    return _NC_CACHE


def _host_prep(inputs):
    x = np.asarray(inputs["x"], dtype=np.float32)
    Ws = [np.asarray(inputs[n], np.float32) for n in
          ("Wf0", "Wi0", "Wh0", "Wf1", "Wi1", "Wh1")]
    bs = [np.asarray(inputs[n], np.float32) for n in
          ("bf0", "bi0", "bh0", "bf1", "bi1", "bh1")]
    bf0, bi0, bh0, bf1, bi1, bh1 = bs
    Wf1, Wi1, Wh1 = Ws[3], Ws[4], Ws[5]
    # h-bias elimination: layer-2 gate biases absorb Wx1 @ bh0
    bf1e = bf1 + Wf1 @ bh0
    bi1e = bi1 + Wi1 @ bh0
    bh1e = bh1 + Wh1 @ bh0

    # bf16 h-gate weights only
    wt_all = np.ascontiguousarray(
        np.stack([Ws[2].T, Ws[5].T])).astype(ml_dtypes.bfloat16)  # [2, din, dout]
    # fp8 f/i weights, scaled by WS (undone via the sigmoid input scale) to
    # keep them out of the fp8 subnormal range; [l*2+gate, p, c, i, dout]
    np_fp8 = mybir.dt.np(mybir.dt.float8e4)
    wfi = np.stack([
        (Ws[j].T * WS).reshape(2, 2, 128, D).transpose(2, 0, 1, 3)
        for j in (0, 1, 3, 4)
    ]).astype(np_fp8)

    bias = np.zeros((128, 2, 8), np.float32)
    gi = np.zeros((128, 2, G), np.float32)
    for g in range(G):
        sl = slice(g * 128, (g + 1) * 128)
        bias[:, 0, 0 * 4 + g] = bf0[sl]
        bias[:, 0, 1 * 4 + g] = bi0[sl]
        bias[:, 1, 0 * 4 + g] = bf1e[sl]
        bias[:, 1, 1 * 4 + g] = bi1e[sl]
        gi[:, 0, g] = -bh0[sl]
        gi[:, 1, g] = -bh1e[sl]
    # min-gate trick requires |bias| < 0.5 <= r = 1/(f+i)
    assert np.abs(bias).max() < 0.45, "bias magnitude breaks min-gate trick"

    w_out = np.asarray(inputs["W_out"], np.float32).reshape(D)
    ln_g = np.asarray(inputs["ln_g"], np.float32)
    ln_b = np.asarray(inputs["ln_b"], np.float32)
    b_out = float(np.asarray(inputs["b_out"], np.float32).reshape(()))
    wg = w_out * ln_g
    c = bh1e                         # constant channel shift of res
    c0 = float(w_out @ ln_b) + b_out
    swg = float(wg.sum())
    C1 = float(c.sum())
    C2 = float((c * c).sum())
    C3 = float((wg * c).sum())
    A = swg / D
    Kc = C1 * A - C3
    epsP = LN_EPS + C2 / D
    C1D = C1 / D

    slt = np.zeros((G, NT, 128, 72), np.float32)
    for g in range(G):
        sl = slice(g * 128, (g + 1) * 128)
        for t in range(NT):
            slt[g, t, :, t] = 1.0
            slt[g, t, :, 32 + t] = wg[sl]
            slt[g, t, :, 64 + t] = c[sl]
    slt = slt.astype(ml_dtypes.bfloat16)

    epi = np.zeros((NT, 8), np.float32)
    epi[:, 0] = c0
    epi[:, 1] = A
    epi[:, 2] = epsP
    epi[:, 3] = C1D
    epi[:, 4] = Kc

    xt = x.transpose(0, 2, 1)                              # [B, D, T]
    xt_b = np.ascontiguousarray(xt).astype(ml_dtypes.bfloat16)
    # fp8 x chunks for DoubleRow: [B, c, p, i, t], channel = c*256+i*128+p
    xf8 = np.ascontiguousarray(
        xt.reshape(B, 2, 2, 128, T).transpose(0, 1, 3, 2, 4)).astype(np_fp8)
    return xt_b, xf8, wt_all, wfi, bias, gi, slt, epi


def _in_maps(inputs):
    xt_b, xf8, wt_all, wfi, bias, gi, slt, epi = _host_prep(inputs)
    return [
        {
            "xt": xt_b[b], "xf8": xf8[b],
            "wt": wt_all, "wfi": wfi, "bias": bias, "gi": gi,
            "slt": slt, "epi": epi,
        }
        for b in range(B)
    ]


def kernel(**inputs):
    nc = _get_nc()
    res = run_bass_kernel_spmd(nc, _in_maps(inputs), list(range(B)))
    out = np.stack([res.results[b]["out"].reshape(T, OUT) for b in range(B)])
    return out.astype(np.float32)


def kernel_traced(**inputs):
    """same as kernel() but returns (output, BassKernelResults) with timing"""
    nc = _get_nc()
    res = run_bass_kernel_spmd(nc, _in_maps(inputs), list(range(B)), trace=True)
    out = np.stack([res.results[b]["out"].reshape(T, OUT) for b in range(B)])
    return out.astype(np.float32), res


# revision 42
# speedup vs baseline: 1.0524x; 1.0447x over previous
"""Trainium2 Bass kernel for nn_DecoderMinLSTMGNN.

Model (per sample): two MinLSTM layers (D=512) over T=4096 steps, residual,
LayerNorm, projection D->1.  B=8 samples are data-parallel across the 8
NeuronCores (one sample per core).  344.5us baseline -> ~240us.

Layout is channels-major: x^T [D, T].  The time-dim linear recurrence
h_t = a_t*h_{t-1} + (1-a_t)*htilde_t runs on the VectorE TensorTensorScan
(one independent recurrence per partition, scanned along the free dim).

Key optimizations:
- h-gate bias elimination (g = h - bh substitution): the recurrence becomes
  bias-free (init -bh); the bias folds into the next layer's f/i gate
  biases (bf1_eff = bf1 + Wf1 @ bh0) and into the LN/projection stats via
  an extra lhsT column (c = bh1_eff) + host-side constants.  Removes all
  64 bias matmuls.
- fp8 (e4m3) DoubleRow matmuls for the f/i gate projections (half the
  cycles/row of bf16): weights are scaled by 64 into fp8's normal range
  and the scale is undone for free via the sigmoid's input-scale field.
  Gate noise is strongly damped (sigmoid slope, a in (0,1), contractive
  scan), so fp8 costs <1e-3 of final rel-err.  The h-gate (htilde path)
  and the LN stats stay bf16; PSUM and scan state are fp32.
- ScalarE act-table phase batching: sigmoid and reciprocal live in
  different act-table sets (1.3us per reload; naive interleave costs ~97
  loads).  Work is organized in (layer, half-of-T) phases: 32 sigmoids,
  then 4 wide reciprocals.  Two artificial-dependency tricks keep the
  scheduler from interleaving phases, both numerically exact:
    min-gate:  next phase's biases pass through min(bias, r) with
               |bias| < 0.45 <= 0.5 <= r = 1/(f+i);
    max-gate:  den[:,0:1] = max(den[:,0:1], i_last) since den = f+i >= i.
  Result: 9 table loads total.
- Deferred scans: each phase's 4 wide scans (the serial DVE hub) are
  emitted after the NEXT sig phase so the priority-heap scheduler uses
  them as gap filler instead of queueing dens behind them.
- bf16 everywhere else (DVE 2x tensor-tensor mode, half SBUF/DMA) and
  wide [128,1024/2048] instructions to amortize per-instruction overhead.
- SBUF slot aliasing (f/r share a ring, den/a share a ring, g2 reuses the
  g1 half-0 slots, gf8 reuses the xf8 slots) to fit in 208KB/partition.
- Epilogue: res/square/stats matmuls interleaved per channel-group with
  the layer-2 scans; LN + projection collapse into 3 matmul-accumulated
  row-stats (s1, s3 = wg.res, sc = c.res, s2 = res^2) + a short fp32
  epilogue on [8,512] tiles.
"""

import numpy as np
import ml_dtypes

import concourse.bass as bass
import concourse.mybir as mybir
import concourse.tile as tile
from concourse.bass_utils import run_bass_kernel_spmd

F32 = mybir.dt.float32
BF16 = mybir.dt.bfloat16
FP8 = mybir.dt.float8e4
DR = mybir.MatmulPerfMode.DoubleRow
WS = 64.0
AF = mybir.ActivationFunctionType
OP = mybir.AluOpType

B, T, D = 8, 4096, 512
OUT = 1
LN_EPS = 1e-5
TT = 512                 # time-tile size
NT = T // TT             # 8 time tiles
G = D // 128             # 4 channel groups
K = D // 128             # 4 contraction chunks
TPH = 4                  # time tiles per phase (half)
HALF = TPH * TT          # 2048
NH = NT // TPH           # 2 halves

MAX_WAITS = 1


def _split_excess_waits(nc):
    """walrus in this container rejects >1 semaphore wait per instruction
    ("Too many sync wait commands"); move excess waits onto NoOps."""
    for fn in nc.m.functions:
        for bb in fn.blocks:
            new_list = []
            changed = False
            for inst in bb.instructions:
                si = inst.sync_info
                waits = list(si.on_wait) if si is not None and si.on_wait else []
                if len(waits) > MAX_WAITS:
                    changed = True
                    overflow = waits[:-MAX_WAITS]
                    si.on_wait = waits[-MAX_WAITS:]
                    for j in range(0, len(overflow), MAX_WAITS):
                        new_list.append(mybir.InstNoOp(
                            name=f"{inst.name}-waitsplit-{j}",
                            engine=inst.engine,
                            ins=[], outs=[],
                            sync_info=mybir.SyncInfo(
                                on_wait=overflow[j:j + MAX_WAITS], on_update=[]),
                        ))
                new_list.append(inst)
            if changed:
                bb.instructions[:] = new_list
    return nc


def _act_direct(nc, out, in_, func, bias=0.0, scale=1.0):
    """emit InstActivation directly (bass blocks Reciprocal/Rsqrt)."""
    ins = [nc.scalar.lower_ap(in_)]
    for v in (bias, scale, 0.0):
        if isinstance(v, (int, float)):
            ins.append(mybir.ImmediateValue(dtype=mybir.dt.float32, value=float(v)))
        else:
            ins.append(nc.scalar.lower_ap(v))
    return nc.scalar.add_instruction(
        mybir.InstActivation(
            name=nc.get_next_instruction_name(),
            func=func, ins=ins, outs=[nc.scalar.lower_ap(out)]))


def _build_nc():
    nc = bass.Bass()

    xt_d = nc.dram_tensor("xt", [D, T], BF16, kind="ExternalInput")
    # fp8 x for the f/i gate matmuls (DoubleRow): [c, p, i, t], ch = c*256+i*128+p
    xf8_d = nc.dram_tensor("xf8", [2, 128, 2, T], FP8, kind="ExternalInput")
    # fp8 f/i weights (x WS), [layer*2+gate, p, c, i, dout]
    wfi_d = nc.dram_tensor("wfi", [4, 128, 2, 2, D], FP8, kind="ExternalInput")
    # bf16 h-gate weights only: [layer, din, dout]
    wt_d = nc.dram_tensor("wt", [2, D, D], BF16, kind="ExternalInput")
    # f/i gate biases (layer-2 ones pre-corrected): [128, layer, gate*4+g]
    bias_d = nc.dram_tensor("bias", [128, 2, 8], F32, kind="ExternalInput")
    # scan initial state columns (-bh_eff): [128, layer, g]
    gi_d = nc.dram_tensor("gi", [128, 2, G], F32, kind="ExternalInput")
    # stats lhsT per (g,t): col t = 1 (s1), col 32+t = wg, col 64+t = c
    slt_d = nc.dram_tensor("slt", [G, NT, 128, 72], BF16, kind="ExternalInput")
    epi_d = nc.dram_tensor("epi", [NT, 8], F32, kind="ExternalInput")
    out_d = nc.dram_tensor("out", [NT, TT], F32, kind="ExternalOutput")

    with tile.TileContext(nc) as tc:
        with (
            tc.tile_pool(name="const", bufs=1) as const,
            tc.tile_pool(name="xtp", bufs=1) as xtp,
            tc.tile_pool(name="gp", bufs=1) as gp,        # wide per-g phase bufs
            tc.tile_pool(name="work", bufs=2) as work,    # i tiles
            tc.tile_pool(name="wk2", bufs=2) as wk2,      # res/sq/bgate/carry
            tc.tile_pool(name="fin", bufs=4) as fin,
            tc.tile_pool(name="gates_ps", bufs=3, space="PSUM") as gates_ps,
            tc.tile_pool(name="stats_ps", bufs=1, space="PSUM") as stats_ps,
        ):
            # ---- constants + x, DMA-ordered so phase (0,0) starts ASAP ----
            wt_sb = [None] * 2
            wfi_sb = [None] * 4
            def _load_wh(idx):
                w = const.tile([128, K, D], BF16, tag=f"wt{idx}", name=f"wt{idx}")
                nc.sync.dma_start(
                    out=w[:], in_=wt_d[idx].rearrange("(k p) d -> p k d", p=128))
                wt_sb[idx] = w
            def _load_wfi(idx):
                w = const.tile([128, 2, 2, D], FP8, tag=f"wfi{idx}", name=f"wfi{idx}")
                nc.sync.dma_start(out=w[:], in_=wfi_d[idx])
                wfi_sb[idx] = w
            _load_wfi(0)
            _load_wfi(1)
            bias_sb = const.tile([128, 2, 8], F32)
            nc.sync.dma_start(out=bias_sb[:], in_=bias_d[:])
            xf8_sb = []
            W2F = 2 * TT
            for c in range(2):
                xc = xtp.tile([128, 2, T], FP8, tag=f"xf8{c}", name=f"xf8{c}")
                nc.gpsimd.dma_start(out=xc[:, :, 0:W2F], in_=xf8_d[c, :, :, 0:W2F])
                xf8_sb.append(xc)
            for c in range(2):
                nc.gpsimd.dma_start(
                    out=xf8_sb[c][:, :, W2F:HALF], in_=xf8_d[c, :, :, W2F:HALF])
            for c in range(2):
                nc.gpsimd.dma_start(
                    out=xf8_sb[c][:, :, HALF:T], in_=xf8_d[c, :, :, HALF:T])
            xt_sb = []
            for g in range(G):
                xx = xtp.tile([128, T], BF16, tag=f"xt{g}", name=f"xt{g}")
                xt_sb.append(xx)
            for h in range(NH):
                for g in range(G):
                    nc.gpsimd.dma_start(
                        out=xt_sb[g][:, h * HALF:(h + 1) * HALF],
                        in_=xt_d[g * 128:(g + 1) * 128, h * HALF:(h + 1) * HALF])
            _load_wh(0)
            gi_sb = const.tile([128, 2, G], F32)
            nc.sync.dma_start(out=gi_sb[:], in_=gi_d[:])
            _load_wfi(2)
            _load_wfi(3)
            _load_wh(1)
            slt_sb = const.tile([128, G, NT, 72], BF16)
            nc.sync.dma_start(
                out=slt_sb[:], in_=slt_d.rearrange("g t p c -> p g t c"))
            epi_sb = const.tile([NT, 8], F32)
            nc.sync.dma_start(out=epi_sb[:], in_=epi_d[:])
            # fp8 copies of g1 for the layer-2 f/i matmuls; alias the xf8
            # slots (xf8 is dead after the last layer-0 f/i matmul).
            gf8_sb = [None, None]

            # layer-1 scan outputs (bf16), resident per (g, half)
            g1_sb = [[None] * NH for _ in range(G)]
            # persistent stats accumulators (PSUM)
            s13_ps = stats_ps.tile([72, TT], F32, tag="s13")
            s2_ps = stats_ps.tile([NT, TT], F32, tag="s2")
            stats_first = [True]
            stats_count = [0]
            N_STATS = G * NT         # stats matmul pairs = 32

            def sig_phase(layer, t0, nt, gate_r):
                """pf/pi matmuls + sigmoids + den for tiles [t0, t0+nt).
                gate_r: previous phase's reciprocal tiles (or None), min-
                gated into the biases (forces ScalarE phase ordering)."""
                if gate_r is None:
                    bsrc = lambda gate, g: bias_sb[:, layer, 4 * gate + g:4 * gate + g + 1]
                else:
                    # high priority: this tiny DVE op gates the whole next
                    # sigmoid block; without it the scheduler queues it
                    # behind ~6us of up-STTs.
                    with tc.high_priority():
                        bg = wk2.tile([128, 8], F32, tag="bgate")
                        nc.vector.tensor_tensor(
                            bg[:], bias_sb[:, layer], gate_r[0][:, 0:8], OP.min)
                        bg2 = wk2.tile([128, 8], F32, tag="bgate", name="bg2")
                        nc.vector.tensor_tensor(
                            bg2[:], bg[:], gate_r[G - 1][:, 0:8], OP.min)
                        bg = bg2
                    bsrc = lambda gate, g: bg[:, 4 * gate + g:4 * gate + g + 1]
                f_t, den_t = [], []
                for g in range(G):
                    f_t.append(gp.tile([128, nt * TT], BF16, tag=f"fr{g}", bufs=2, name=f"f{g}"))
                    den_t.append(gp.tile([128, nt * TT], BF16, tag=f"da{g}", bufs=2, name=f"den{g}"))
                src8 = xf8_sb if layer == 0 else gf8_sb
                W2 = 2 * TT
                for tp in range(nt // 2):
                    for g in range(G):
                        pf = gates_ps.tile([128, W2], F32, tag="mm", name="pf")
                        pi = gates_ps.tile([128, W2], F32, tag="mm", name="pi")
                        for gate, ps in ((0, pf), (1, pi)):
                            w8 = wfi_sb[2 * layer + gate]
                            for sub in range(2):
                                t = t0 + tp * 2 + sub
                                for c in range(2):
                                    nc.tensor.matmul(
                                        ps[:, sub * TT:(sub + 1) * TT],
                                        w8[:, c, :, g * 128:(g + 1) * 128],
                                        src8[c][:, :, t * TT:(t + 1) * TT],
                                        start=(c == 0), stop=(c == 1), perf_mode=DR)
                        fs = f_t[g][:, tp * W2:(tp + 1) * W2]
                        nc.scalar.activation(fs, pf[:], AF.Sigmoid, bias=bsrc(0, g),
                                             scale=1.0 / WS)
                        i_sb = work.tile([128, W2], BF16, tag="i")
                        nc.scalar.activation(i_sb[:], pi[:], AF.Sigmoid, bias=bsrc(1, g),
                                             scale=1.0 / WS)
                        nc.vector.tensor_add(
                            den_t[g][:, tp * W2:(tp + 1) * W2], fs, i_sb[:])
                        i_last = i_sb
                # gate all reciprocals on the last sigmoid of the phase:
                # max(den, i) == den exactly (den = f+i >= i), so this only
                # adds the dependency, keeping the act-table phases contiguous.
                for g in range(G):
                    nc.vector.tensor_tensor(
                        den_t[g][:, 0:1], den_t[g][:, 0:1], i_last[:, 0:1], OP.max)
                return f_t, den_t

            def rec_phase(layer, t0, nt, f_t, den_t, g2_carry):
                """reciprocal + a + u' for tiles [t0, t0+nt); scans are
                emitted later (emit_scans closure) so the next sig phase's
                den adds aren't queued behind them on DVE."""
                r_t, a_t, up_t = [], [], []
                # high priority: reciprocals must not queue behind the
                # set-neutral gf8 casts on ScalarE at layer transitions.
                with tc.high_priority():
                    for g in range(G):
                        r = gp.tile([128, nt * TT], BF16, tag=f"fr{g}", bufs=2, name=f"r{g}")
                        _act_direct(nc, r[:], den_t[g][:], AF.Reciprocal)
                        r_t.append(r)
                for g in range(G):
                    a = gp.tile([128, nt * TT], BF16, tag=f"da{g}", bufs=2, name=f"a{g}")
                    nc.vector.tensor_mul(a[:], f_t[g][:], r_t[g][:])
                    a_t.append(a)
                    up_t.append(gp.tile([128, nt * TT], BF16, tag=f"up{g}", bufs=1, name=f"up{g}"))
                W2 = 2 * TT
                def zh_pair(tp):
                    for g in range(G):
                        ph = gates_ps.tile([128, W2], F32, tag="mm", name="ph")
                        w = wt_sb[layer]
                        for sub in range(2):
                            t = t0 + tp * 2 + sub
                            for k in range(K):
                                if layer == 0:
                                    r = xt_sb[k][:, t * TT:(t + 1) * TT]
                                else:
                                    r = g1_sb[k][t // TPH][:, (t % TPH) * TT:(t % TPH + 1) * TT]
                                nc.tensor.matmul(
                                    ph[:, sub * TT:(sub + 1) * TT],
                                    w[:, k, g * 128:(g + 1) * 128], r,
                                    start=(k == 0), stop=(k == K - 1))
                        nc.vector.scalar_tensor_tensor(
                            up_t[g][:, tp * W2:(tp + 1) * W2],
                            a_t[g][:, tp * W2:(tp + 1) * W2], 1.0, ph[:],
                            OP.subtract, OP.mult)
                for tp in range(nt // 2):
                    zh_pair(tp)
                def emit_zh_last():
                    pass
                def emit_scans(epi_g=None):
                    gout = []
                    carry = [] if (layer == 1 and t0 + nt < NT) else None
                    for g in range(G):
                        if layer == 0:
                            half = t0 // TPH
                            go = gp.tile([128, nt * TT], BF16, tag=f"g1_{g}_{half}", name=f"g1_{g}_{half}")
                            init = (gi_sb[:, 0, g:g + 1] if t0 == 0
                                    else g1_sb[g][0][:, HALF - 1:HALF])
                            g1_sb[g][half] = go
                        else:
                            go = gp.tile([128, nt * TT], BF16, tag=f"g1_{g}_0", name=f"g2_{g}_{t0}")
                            init = (gi_sb[:, 1, g:g + 1] if t0 == 0
                                    else g2_carry[g][:])
                        nc.vector.tensor_tensor_scan(
                            go[:], a_t[g][:], up_t[g][:], init, OP.mult, OP.subtract)
                        gout.append(go)
                        if layer == 0:
                            c8, i8 = g // 2, g % 2
                            if gf8_sb[c8] is None:
                                gf8_sb[c8] = xtp.tile(
                                    [128, 2, T], FP8, tag=f"xf8{c8}", name=f"gf8{c8}")
                            nc.scalar.activation(
                                gf8_sb[c8][:, i8, t0 * TT:(t0 + nt) * TT],
                                go[:], AF.Copy)
                        if carry is not None:
                            cr = wk2.tile([128, 1], BF16, tag=f"carry{g}", name=f"carry{g}")
                            nc.vector.tensor_copy(cr[:], go[:, nt * TT - 1:nt * TT])
                            carry.append(cr)
                        if epi_g is not None:
                            epi_g(g, go)
                    return gout, carry
                return r_t, emit_scans, emit_zh_last

            def epilogue(t0, nt):
                """per-g closure: res = g2 + x^T, square, stats matmuls."""
                def epi_g(g, g2):
                    res = wk2.tile([128, nt * TT], BF16, tag="res", bufs=1)
                    nc.vector.tensor_add(
                        res[:], g2[:],
                        xt_sb[g][:, t0 * TT:(t0 + nt) * TT])
                    sq = wk2.tile([128, nt * TT], BF16, tag="sq", bufs=1)
                    nc.scalar.activation(sq[:], res[:], AF.Square)
                    for ti in range(nt):
                        t = t0 + ti
                        rs = res[:, ti * TT:(ti + 1) * TT]
                        sqs = sq[:, ti * TT:(ti + 1) * TT]
                        first = stats_first[0]
                        stats_first[0] = False
                        stats_count[0] += 1
                        last = stats_count[0] == N_STATS
                        nc.tensor.matmul(
                            s13_ps[:], slt_sb[:, g, t, 0:72], rs,
                            start=first, stop=last, skip_group_check=True)
                        nc.tensor.matmul(
                            s2_ps[:], slt_sb[:, g, t, 0:8], sqs,
                            start=first, stop=last, skip_group_check=True)
                return epi_g

            # ---- pipeline (scans deferred past the next sig phase);
            #      layer-1 split [0-3],[4-5],[6-7] to shrink the tail ----
            f_t, den_t = sig_phase(0, 0, 4, None)
            r_a, sc_a, zh_a = rec_phase(0, 0, 4, f_t, den_t, None)
            f_t, den_t = sig_phase(0, 4, 4, r_a)
            zh_a()
            sc_a()
            r_b, sc_b, zh_b = rec_phase(0, 4, 4, f_t, den_t, None)
            f_t, den_t = sig_phase(1, 0, 4, r_b)
            zh_b()
            sc_b()
            r_c, sc_c, zh_c = rec_phase(1, 0, 4, f_t, den_t, None)
            f_t, den_t = sig_phase(1, 4, 2, r_c)
            zh_c()
            _, carry1 = sc_c(epilogue(0, 4))
            r_d, sc_d, zh_d = rec_phase(1, 4, 2, f_t, den_t, carry1)
            f_t, den_t = sig_phase(1, 6, 2, r_d)
            zh_d()
            _, carry2 = sc_d(epilogue(4, 2))
            r_e, sc_e, zh_e = rec_phase(1, 6, 2, f_t, den_t, carry2)
            zh_e()
            # preload the rsqrt act-table during the last scan window
            # (gated on the final reciprocal so it can't reorder earlier)
            rsq_pre = wk2.tile([NT, 1], F32, tag="rsqpre")
            _act_direct(nc, rsq_pre[:], r_e[G - 1][0:NT, 0:1], AF.Rsqrt)
            sc_e(epilogue(6, 2))

            # ---- final LN + projection math on [8, 512] ----
            # y = -( (s1_0*A - s3_0) + Kc ) * rsqrt(v + eps') + c0
            # v  = (s2_0 + 2*sc)/D - ((s1_0 + C1)/D)^2
            sc_sb = fin.tile([NT, TT], F32, tag="fin")
            nc.scalar.activation(sc_sb[:], s13_ps[64:64 + NT, :], AF.Copy)
            s3_sb = fin.tile([NT, TT], F32, tag="fin")
            nc.scalar.activation(s3_sb[:], s13_ps[32:32 + NT, :], AF.Copy)
            s2c = fin.tile([NT, TT], F32, tag="fin")
            nc.vector.scalar_tensor_tensor(
                s2c[:], sc_sb[:], 2.0, s2_ps[:], OP.mult, OP.add)
            mu2 = fin.tile([NT, TT], F32, tag="fin")
            nc.scalar.activation(mu2[:], s13_ps[0:NT, :], AF.Square,
                                 bias=epi_sb[:, 3:4], scale=1.0 / D)
            v = fin.tile([NT, TT], F32, tag="fin")
            nc.vector.scalar_tensor_tensor(
                v[:], s2c[:], 1.0 / D, mu2[:], OP.mult, OP.subtract)
            rv = fin.tile([NT, TT], F32, tag="fin")
            _act_direct(nc, rv[:], v[:], AF.Rsqrt, bias=epi_sb[:, 2:3])
            q = fin.tile([NT, TT], F32, tag="fin")
            nc.vector.scalar_tensor_tensor(
                q[:], s13_ps[0:NT, :], epi_sb[:, 1:2], s3_sb[:],
                OP.mult, OP.subtract)
            z = fin.tile([NT, TT], F32, tag="fin")
            nc.vector.scalar_tensor_tensor(
                z[:], q[:], epi_sb[:, 4:5], rv[:], OP.add, OP.mult)
            o_sb = fin.tile([NT, TT], F32, tag="fin")
            nc.scalar.activation(o_sb[:], z[:], AF.Identity,
                                 bias=epi_sb[:, 0:1], scale=-1.0)
            nc.sync.dma_start(out=out_d[:], in_=o_sb[:])

    _split_excess_waits(nc)
    return nc


_NC_CACHE = None


def _get_nc():
    global _NC_CACHE
    if _NC_CACHE is None:
        _NC_CACHE = _build_nc()
    return _NC_CACHE


def _host_prep(inputs):
    x = np.asarray(inputs["x"], dtype=np.float32)
    Ws = [np.asarray(inputs[n], np.float32) for n in
          ("Wf0", "Wi0", "Wh0", "Wf1", "Wi1", "Wh1")]
    bs = [np.asarray(inputs[n], np.float32) for n in
          ("bf0", "bi0", "bh0", "bf1", "bi1", "bh1")]
    bf0, bi0, bh0, bf1, bi1, bh1 = bs
    Wf1, Wi1, Wh1 = Ws[3], Ws[4], Ws[5]
    # h-bias elimination: layer-2 gate biases absorb Wx1 @ bh0
    bf1e = bf1 + Wf1 @ bh0
    bi1e = bi1 + Wi1 @ bh0
    bh1e = bh1 + Wh1 @ bh0

    # bf16 h-gate weights only
    wt_all = np.ascontiguousarray(
        np.stack([Ws[2].T, Ws[5].T])).astype(ml_dtypes.bfloat16)  # [2, din, dout]
    # fp8 f/i weights, scaled by WS (undone via the sigmoid input scale) to
    # keep them out of the fp8 subnormal range; [l*2+gate, p, c, i, dout]
    np_fp8 = mybir.dt.np(mybir.dt.float8e4)
    wfi = np.stack([
        (Ws[j].T * WS).reshape(2, 2, 128, D).transpose(2, 0, 1, 3)
        for j in (0, 1, 3, 4)
    ]).astype(np_fp8)

    bias = np.zeros((128, 2, 8), np.float32)
    gi = np.zeros((128, 2, G), np.float32)
    for g in range(G):
        sl = slice(g * 128, (g + 1) * 128)
        bias[:, 0, 0 * 4 + g] = bf0[sl]
        bias[:, 0, 1 * 4 + g] = bi0[sl]
        bias[:, 1, 0 * 4 + g] = bf1e[sl]
        bias[:, 1, 1 * 4 + g] = bi1e[sl]
        gi[:, 0, g] = -bh0[sl]
        gi[:, 1, g] = -bh1e[sl]
    # min-gate trick requires |bias| < 0.5 <= r = 1/(f+i)
    assert np.abs(bias).max() < 0.45, "bias magnitude breaks min-gate trick"

    w_out = np.asarray(inputs["W_out"], np.float32).reshape(D)
    ln_g = np.asarray(inputs["ln_g"], np.float32)
    ln_b = np.asarray(inputs["ln_b"], np.float32)
    b_out = float(np.asarray(inputs["b_out"], np.float32).reshape(()))
    wg = w_out * ln_g
    c = bh1e                         # constant channel shift of res
    c0 = float(w_out @ ln_b) + b_out
    swg = float(wg.sum())
    C1 = float(c.sum())
    C2 = float((c * c).sum())
    C3 = float((wg * c).sum())
    A = swg / D
    Kc = C1 * A - C3
    epsP = LN_EPS + C2 / D
    C1D = C1 / D

    slt = np.zeros((G, NT, 128, 72), np.float32)
    for g in range(G):
        sl = slice(g * 128, (g + 1) * 128)
        for t in range(NT):
            slt[g, t, :, t] = 1.0
            slt[g, t, :, 32 + t] = wg[sl]
            slt[g, t, :, 64 + t] = c[sl]
    slt = slt.astype(ml_dtypes.bfloat16)

    epi = np.zeros((NT, 8), np.float32)
    epi[:, 0] = c0
    epi[:, 1] = A
    epi[:, 2] = epsP
    epi[:, 3] = C1D
    epi[:, 4] = Kc

    xt = x.transpose(0, 2, 1)                              # [B, D, T]
    xt_b = np.ascontiguousarray(xt).astype(ml_dtypes.bfloat16)
    # fp8 x chunks for DoubleRow: [B, c, p, i, t], channel = c*256+i*128+p
    xf8 = np.ascontiguousarray(
        xt.reshape(B, 2, 2, 128, T).transpose(0, 1, 3, 2, 4)).astype(np_fp8)
    return xt_b, xf8, wt_all, wfi, bias, gi, slt, epi


def _in_maps(inputs):
    xt_b, xf8, wt_all, wfi, bias, gi, slt, epi = _host_prep(inputs)
    return [
        {
            "xt": xt_b[b], "xf8": xf8[b],
            "wt": wt_all, "wfi": wfi, "bias": bias, "gi": gi,
            "slt": slt, "epi": epi,
        }
        for b in range(B)
    ]


def kernel(**inputs):
    nc = _get_nc()
    res = run_bass_kernel_spmd(nc, _in_maps(inputs), list(range(B)))
    out = np.stack([res.results[b]["out"].reshape(T, OUT) for b in range(B)])
    return out.astype(np.float32)


def kernel_traced(**inputs):
    """same as kernel() but returns (output, BassKernelResults) with timing"""
    nc = _get_nc()
    res = run_bass_kernel_spmd(nc, _in_maps(inputs), list(range(B)), trace=True)
    out = np.stack([res.results[b]["out"].reshape(T, OUT) for b in range(B)])
    return out.astype(np.float32), res


# revision 43
# speedup vs baseline: 1.0545x; 1.0019x over previous
"""Trainium2 Bass kernel for nn_DecoderMinLSTMGNN.

Model (per sample): two MinLSTM layers (D=512) over T=4096 steps, residual,
LayerNorm, projection D->1.  B=8 samples are data-parallel across the 8
NeuronCores (one sample per core).  344.5us baseline -> ~240us.

Layout is channels-major: x^T [D, T].  The time-dim linear recurrence
h_t = a_t*h_{t-1} + (1-a_t)*htilde_t runs on the VectorE TensorTensorScan
(one independent recurrence per partition, scanned along the free dim).

Key optimizations:
- h-gate bias elimination (g = h - bh substitution): the recurrence becomes
  bias-free (init -bh); the bias folds into the next layer's f/i gate
  biases (bf1_eff = bf1 + Wf1 @ bh0) and into the LN/projection stats via
  an extra lhsT column (c = bh1_eff) + host-side constants.  Removes all
  64 bias matmuls.
- fp8 (e4m3) DoubleRow matmuls for the f/i gate projections (half the
  cycles/row of bf16): weights are scaled by 64 into fp8's normal range
  and the scale is undone for free via the sigmoid's input-scale field.
  Gate noise is strongly damped (sigmoid slope, a in (0,1), contractive
  scan), so fp8 costs <1e-3 of final rel-err.  The h-gate (htilde path)
  and the LN stats stay bf16; PSUM and scan state are fp32.
- ScalarE act-table phase batching: sigmoid and reciprocal live in
  different act-table sets (1.3us per reload; naive interleave costs ~97
  loads).  Work is organized in (layer, half-of-T) phases: 32 sigmoids,
  then 4 wide reciprocals.  Two artificial-dependency tricks keep the
  scheduler from interleaving phases, both numerically exact:
    min-gate:  next phase's biases pass through min(bias, r) with
               |bias| < 0.45 <= 0.5 <= r = 1/(f+i);
    max-gate:  den[:,0:1] = max(den[:,0:1], i_last) since den = f+i >= i.
  Result: 9 table loads total.
- Deferred scans: each phase's 4 wide scans (the serial DVE hub) are
  emitted after the NEXT sig phase so the priority-heap scheduler uses
  them as gap filler instead of queueing dens behind them.
- bf16 everywhere else (DVE 2x tensor-tensor mode, half SBUF/DMA) and
  wide [128,1024/2048] instructions to amortize per-instruction overhead.
- SBUF slot aliasing (f/r share a ring, den/a share a ring, g2 reuses the
  g1 half-0 slots, gf8 reuses the xf8 slots) to fit in 208KB/partition.
- Epilogue: res/square/stats matmuls interleaved per channel-group with
  the layer-2 scans; LN + projection collapse into 3 matmul-accumulated
  row-stats (s1, s3 = wg.res, sc = c.res, s2 = res^2) + a short fp32
  epilogue on [8,512] tiles.
"""

import numpy as np
import ml_dtypes

import concourse.bass as bass
import concourse.mybir as mybir
import concourse.tile as tile
from concourse.bass_utils import run_bass_kernel_spmd

F32 = mybir.dt.float32
BF16 = mybir.dt.bfloat16
FP8 = mybir.dt.float8e4
DR = mybir.MatmulPerfMode.DoubleRow
WS = 64.0
AF = mybir.ActivationFunctionType
OP = mybir.AluOpType

B, T, D = 8, 4096, 512
OUT = 1
LN_EPS = 1e-5
TT = 512                 # time-tile size
NT = T // TT             # 8 time tiles
G = D // 128             # 4 channel groups
K = D // 128             # 4 contraction chunks
TPH = 4                  # time tiles per phase (half)
HALF = TPH * TT          # 2048
NH = NT // TPH           # 2 halves

MAX_WAITS = 1


def _split_excess_waits(nc):
    """walrus in this container rejects >1 semaphore wait per instruction
    ("Too many sync wait commands"); move excess waits onto NoOps."""
    for fn in nc.m.functions:
        for bb in fn.blocks:
            new_list = []
            changed = False
            for inst in bb.instructions:
                si = inst.sync_info
                waits = list(si.on_wait) if si is not None and si.on_wait else []
                if len(waits) > MAX_WAITS:
                    changed = True
                    overflow = waits[:-MAX_WAITS]
                    si.on_wait = waits[-MAX_WAITS:]
                    for j in range(0, len(overflow), MAX_WAITS):
                        new_list.append(mybir.InstNoOp(
                            name=f"{inst.name}-waitsplit-{j}",
                            engine=inst.engine,
                            ins=[], outs=[],
                            sync_info=mybir.SyncInfo(
                                on_wait=overflow[j:j + MAX_WAITS], on_update=[]),
                        ))
                new_list.append(inst)
            if changed:
                bb.instructions[:] = new_list
    return nc


def _act_direct(nc, out, in_, func, bias=0.0, scale=1.0):
    """emit InstActivation directly (bass blocks Reciprocal/Rsqrt)."""
    ins = [nc.scalar.lower_ap(in_)]
    for v in (bias, scale, 0.0):
        if isinstance(v, (int, float)):
            ins.append(mybir.ImmediateValue(dtype=mybir.dt.float32, value=float(v)))
        else:
            ins.append(nc.scalar.lower_ap(v))
    return nc.scalar.add_instruction(
        mybir.InstActivation(
            name=nc.get_next_instruction_name(),
            func=func, ins=ins, outs=[nc.scalar.lower_ap(out)]))


def _build_nc():
    nc = bass.Bass()

    xt_d = nc.dram_tensor("xt", [D, T], BF16, kind="ExternalInput")
    # fp8 x for the f/i gate matmuls (DoubleRow): [c, p, i, t], ch = c*256+i*128+p
    xf8_d = nc.dram_tensor("xf8", [2, 128, 2, T], FP8, kind="ExternalInput")
    # fp8 f/i weights (x WS), [layer*2+gate, p, c, i, dout]
    wfi_d = nc.dram_tensor("wfi", [4, 128, 2, 2, D], FP8, kind="ExternalInput")
    # bf16 h-gate weights only: [layer, din, dout]
    wt_d = nc.dram_tensor("wt", [2, D, D], BF16, kind="ExternalInput")
    # f/i gate biases (layer-2 ones pre-corrected): [128, layer, gate*4+g]
    bias_d = nc.dram_tensor("bias", [128, 2, 8], F32, kind="ExternalInput")
    # scan initial state columns (-bh_eff): [128, layer, g]
    gi_d = nc.dram_tensor("gi", [128, 2, G], F32, kind="ExternalInput")
    # stats lhsT per (g,t): col t = 1 (s1), col 32+t = wg, col 64+t = c
    slt_d = nc.dram_tensor("slt", [G, NT, 128, 72], BF16, kind="ExternalInput")
    epi_d = nc.dram_tensor("epi", [NT, 8], F32, kind="ExternalInput")
    out_d = nc.dram_tensor("out", [NT, TT], F32, kind="ExternalOutput")

    with tile.TileContext(nc) as tc:
        with (
            tc.tile_pool(name="const", bufs=1) as const,
            tc.tile_pool(name="xtp", bufs=1) as xtp,
            tc.tile_pool(name="gp", bufs=1) as gp,        # wide per-g phase bufs
            tc.tile_pool(name="work", bufs=2) as work,    # i tiles
            tc.tile_pool(name="wk2", bufs=2) as wk2,      # res/sq/bgate/carry
            tc.tile_pool(name="fin", bufs=4) as fin,
            tc.tile_pool(name="gates_ps", bufs=3, space="PSUM") as gates_ps,
            tc.tile_pool(name="stats_ps", bufs=1, space="PSUM") as stats_ps,
        ):
            # ---- constants + x, DMA-ordered so phase (0,0) starts ASAP ----
            wt_sb = [None] * 2
            wfi_sb = [None] * 4
            def _load_wh(idx):
                w = const.tile([128, K, D], BF16, tag=f"wt{idx}", name=f"wt{idx}")
                nc.sync.dma_start(
                    out=w[:], in_=wt_d[idx].rearrange("(k p) d -> p k d", p=128))
                wt_sb[idx] = w
            def _load_wfi(idx):
                w = const.tile([128, 2, 2, D], FP8, tag=f"wfi{idx}", name=f"wfi{idx}")
                nc.sync.dma_start(out=w[:], in_=wfi_d[idx])
                wfi_sb[idx] = w
            _load_wfi(0)
            _load_wfi(1)
            bias_sb = const.tile([128, 2, 8], F32)
            nc.sync.dma_start(out=bias_sb[:], in_=bias_d[:])
            xf8_sb = []
            W2F = 2 * TT
            for c in range(2):
                xc = xtp.tile([128, 2, T], FP8, tag=f"xf8{c}", name=f"xf8{c}")
                nc.gpsimd.dma_start(out=xc[:, :, 0:W2F], in_=xf8_d[c, :, :, 0:W2F])
                xf8_sb.append(xc)
            for c in range(2):
                nc.gpsimd.dma_start(
                    out=xf8_sb[c][:, :, W2F:HALF], in_=xf8_d[c, :, :, W2F:HALF])
            for c in range(2):
                nc.gpsimd.dma_start(
                    out=xf8_sb[c][:, :, HALF:T], in_=xf8_d[c, :, :, HALF:T])
            xt_sb = []
            for g in range(G):
                xx = xtp.tile([128, T], BF16, tag=f"xt{g}", name=f"xt{g}")
                xt_sb.append(xx)
            for h in range(NH):
                for g in range(G):
                    nc.gpsimd.dma_start(
                        out=xt_sb[g][:, h * HALF:(h + 1) * HALF],
                        in_=xt_d[g * 128:(g + 1) * 128, h * HALF:(h + 1) * HALF])
            _load_wh(0)
            gi_sb = const.tile([128, 2, G], F32)
            nc.sync.dma_start(out=gi_sb[:], in_=gi_d[:])
            _load_wfi(2)
            _load_wfi(3)
            _load_wh(1)
            slt_sb = const.tile([128, G, NT, 72], BF16)
            nc.sync.dma_start(
                out=slt_sb[:], in_=slt_d.rearrange("g t p c -> p g t c"))
            epi_sb = const.tile([NT, 8], F32)
            nc.sync.dma_start(out=epi_sb[:], in_=epi_d[:])
            # fp8 copies of g1 for the layer-2 f/i matmuls; alias the xf8
            # slots (xf8 is dead after the last layer-0 f/i matmul).
            gf8_sb = [None, None]

            # layer-1 scan outputs (bf16), resident per (g, half)
            g1_sb = [[None] * NH for _ in range(G)]
            # persistent stats accumulators (PSUM)
            s13_ps = stats_ps.tile([72, TT], F32, tag="s13")
            s2_ps = stats_ps.tile([NT, TT], F32, tag="s2")
            stats_first = [True]
            stats_count = [0]
            N_STATS = G * NT         # stats matmul pairs = 32

            def sig_phase(layer, t0, nt, gate_r):
                """pf/pi matmuls + sigmoids + den for tiles [t0, t0+nt).
                gate_r: previous phase's reciprocal tiles (or None), min-
                gated into the biases (forces ScalarE phase ordering)."""
                if gate_r is None:
                    bsrc = lambda gate, g: bias_sb[:, layer, 4 * gate + g:4 * gate + g + 1]
                else:
                    # high priority: this tiny DVE op gates the whole next
                    # sigmoid block; without it the scheduler queues it
                    # behind ~6us of up-STTs.
                    with tc.high_priority():
                        bg = wk2.tile([128, 8], F32, tag="bgate")
                        nc.vector.tensor_tensor(
                            bg[:], bias_sb[:, layer], gate_r[0][:, 0:8], OP.min)
                        bg2 = wk2.tile([128, 8], F32, tag="bgate", name="bg2")
                        nc.vector.tensor_tensor(
                            bg2[:], bg[:], gate_r[G - 1][:, 0:8], OP.min)
                        bg = bg2
                    bsrc = lambda gate, g: bg[:, 4 * gate + g:4 * gate + g + 1]
                f_t, den_t = [], []
                for g in range(G):
                    f_t.append(gp.tile([128, nt * TT], BF16, tag=f"fr{g}", bufs=2, name=f"f{g}"))
                    den_t.append(gp.tile([128, nt * TT], BF16, tag=f"da{g}", bufs=2, name=f"den{g}"))
                src8 = xf8_sb if layer == 0 else gf8_sb
                W2 = 2 * TT
                for tp in range(nt // 2):
                    for g in range(G):
                        pf = gates_ps.tile([128, W2], F32, tag="mm", name="pf")
                        pi = gates_ps.tile([128, W2], F32, tag="mm", name="pi")
                        for gate, ps in ((0, pf), (1, pi)):
                            w8 = wfi_sb[2 * layer + gate]
                            for sub in range(2):
                                t = t0 + tp * 2 + sub
                                for c in range(2):
                                    nc.tensor.matmul(
                                        ps[:, sub * TT:(sub + 1) * TT],
                                        w8[:, c, :, g * 128:(g + 1) * 128],
                                        src8[c][:, :, t * TT:(t + 1) * TT],
                                        start=(c == 0), stop=(c == 1), perf_mode=DR)
                        fs = f_t[g][:, tp * W2:(tp + 1) * W2]
                        nc.scalar.activation(fs, pf[:], AF.Sigmoid, bias=bsrc(0, g),
                                             scale=1.0 / WS)
                        i_sb = work.tile([128, W2], BF16, tag="i")
                        nc.scalar.activation(i_sb[:], pi[:], AF.Sigmoid, bias=bsrc(1, g),
                                             scale=1.0 / WS)
                        with tc.high_priority():
                            nc.vector.tensor_add(
                                den_t[g][:, tp * W2:(tp + 1) * W2], fs, i_sb[:])
                        i_last = i_sb
                # gate all reciprocals on the last sigmoid of the phase:
                # max(den, i) == den exactly (den = f+i >= i), so this only
                # adds the dependency, keeping the act-table phases contiguous.
                with tc.high_priority():
                    for g in range(G):
                        nc.vector.tensor_tensor(
                            den_t[g][:, 0:1], den_t[g][:, 0:1], i_last[:, 0:1], OP.max)
                return f_t, den_t

            def rec_phase(layer, t0, nt, f_t, den_t, g2_carry):
                """reciprocal + a + u' for tiles [t0, t0+nt); scans are
                emitted later (emit_scans closure) so the next sig phase's
                den adds aren't queued behind them on DVE."""
                r_t, a_t, up_t = [], [], []
                # high priority: reciprocals must not queue behind the
                # set-neutral gf8 casts on ScalarE at layer transitions.
                with tc.high_priority():
                    for g in range(G):
                        r = gp.tile([128, nt * TT], BF16, tag=f"fr{g}", bufs=2, name=f"r{g}")
                        _act_direct(nc, r[:], den_t[g][:], AF.Reciprocal)
                        r_t.append(r)
                with tc.high_priority():
                    for g in range(G):
                        a = gp.tile([128, nt * TT], BF16, tag=f"da{g}", bufs=2, name=f"a{g}")
                        nc.vector.tensor_mul(a[:], f_t[g][:], r_t[g][:])
                        a_t.append(a)
                        up_t.append(gp.tile([128, nt * TT], BF16, tag=f"up{g}", bufs=1, name=f"up{g}"))
                W2 = 2 * TT
                def zh_pair(tp):
                    for g in range(G):
                        ph = gates_ps.tile([128, W2], F32, tag="mm", name="ph")
                        w = wt_sb[layer]
                        for sub in range(2):
                            t = t0 + tp * 2 + sub
                            for k in range(K):
                                if layer == 0:
                                    r = xt_sb[k][:, t * TT:(t + 1) * TT]
                                else:
                                    r = g1_sb[k][t // TPH][:, (t % TPH) * TT:(t % TPH + 1) * TT]
                                nc.tensor.matmul(
                                    ph[:, sub * TT:(sub + 1) * TT],
                                    w[:, k, g * 128:(g + 1) * 128], r,
                                    start=(k == 0), stop=(k == K - 1))
                        with tc.high_priority():
                            nc.vector.scalar_tensor_tensor(
                                up_t[g][:, tp * W2:(tp + 1) * W2],
                                a_t[g][:, tp * W2:(tp + 1) * W2], 1.0, ph[:],
                                OP.subtract, OP.mult)
                for tp in range(nt // 2):
                    zh_pair(tp)
                def emit_zh_last():
                    pass
                def emit_scans(epi_g=None):
                    gout = []
                    carry = [] if (layer == 1 and t0 + nt < NT) else None
                    for g in range(G):
                        if layer == 0:
                            half = t0 // TPH
                            go = gp.tile([128, nt * TT], BF16, tag=f"g1_{g}_{half}", name=f"g1_{g}_{half}")
                            init = (gi_sb[:, 0, g:g + 1] if t0 == 0
                                    else g1_sb[g][0][:, HALF - 1:HALF])
                            g1_sb[g][half] = go
                        else:
                            go = gp.tile([128, nt * TT], BF16, tag=f"g1_{g}_0", name=f"g2_{g}_{t0}")
                            init = (gi_sb[:, 1, g:g + 1] if t0 == 0
                                    else g2_carry[g][:])
                        nc.vector.tensor_tensor_scan(
                            go[:], a_t[g][:], up_t[g][:], init, OP.mult, OP.subtract)
                        gout.append(go)
                        if layer == 0:
                            c8, i8 = g // 2, g % 2
                            if gf8_sb[c8] is None:
                                gf8_sb[c8] = xtp.tile(
                                    [128, 2, T], FP8, tag=f"xf8{c8}", name=f"gf8{c8}")
                            nc.scalar.activation(
                                gf8_sb[c8][:, i8, t0 * TT:(t0 + nt) * TT],
                                go[:], AF.Copy)
                        if carry is not None:
                            cr = wk2.tile([128, 1], BF16, tag=f"carry{g}", name=f"carry{g}")
                            nc.vector.tensor_copy(cr[:], go[:, nt * TT - 1:nt * TT])
                            carry.append(cr)
                        if epi_g is not None:
                            epi_g(g, go)
                    return gout, carry
                return r_t, emit_scans, emit_zh_last

            def epilogue(t0, nt):
                """per-g closure: res = g2 + x^T, square, stats matmuls."""
                def epi_g(g, g2):
                    res = wk2.tile([128, nt * TT], BF16, tag="res", bufs=1)
                    nc.vector.tensor_add(
                        res[:], g2[:],
                        xt_sb[g][:, t0 * TT:(t0 + nt) * TT])
                    sq = wk2.tile([128, nt * TT], BF16, tag="sq", bufs=1)
                    nc.scalar.activation(sq[:], res[:], AF.Square)
                    for ti in range(nt):
                        t = t0 + ti
                        rs = res[:, ti * TT:(ti + 1) * TT]
                        sqs = sq[:, ti * TT:(ti + 1) * TT]
                        first = stats_first[0]
                        stats_first[0] = False
                        stats_count[0] += 1
                        last = stats_count[0] == N_STATS
                        nc.tensor.matmul(
                            s13_ps[:], slt_sb[:, g, t, 0:72], rs,
                            start=first, stop=last, skip_group_check=True)
                        nc.tensor.matmul(
                            s2_ps[:], slt_sb[:, g, t, 0:8], sqs,
                            start=first, stop=last, skip_group_check=True)
                return epi_g

            # ---- pipeline (scans deferred past the next sig phase);
            #      layer-1 split [0-3],[4-5],[6-7] to shrink the tail ----
            f_t, den_t = sig_phase(0, 0, 4, None)
            r_a, sc_a, zh_a = rec_phase(0, 0, 4, f_t, den_t, None)
            f_t, den_t = sig_phase(0, 4, 4, r_a)
            zh_a()
            sc_a()
            r_b, sc_b, zh_b = rec_phase(0, 4, 4, f_t, den_t, None)
            f_t, den_t = sig_phase(1, 0, 4, r_b)
            zh_b()
            sc_b()
            r_c, sc_c, zh_c = rec_phase(1, 0, 4, f_t, den_t, None)
            f_t, den_t = sig_phase(1, 4, 2, r_c)
            zh_c()
            _, carry1 = sc_c(epilogue(0, 4))
            r_d, sc_d, zh_d = rec_phase(1, 4, 2, f_t, den_t, carry1)
            f_t, den_t = sig_phase(1, 6, 2, r_d)
            zh_d()
            _, carry2 = sc_d(epilogue(4, 2))
            r_e, sc_e, zh_e = rec_phase(1, 6, 2, f_t, den_t, carry2)
            zh_e()
            # preload the rsqrt act-table during the last scan window
            # (gated on the final reciprocal so it can't reorder earlier)
            rsq_pre = wk2.tile([NT, 1], F32, tag="rsqpre")
            _act_direct(nc, rsq_pre[:], r_e[G - 1][0:NT, 0:1], AF.Rsqrt)
            sc_e(epilogue(6, 2))

            # ---- final LN + projection math on [8, 512] ----
            # y = -( (s1_0*A - s3_0) + Kc ) * rsqrt(v + eps') + c0
            # v  = (s2_0 + 2*sc)/D - ((s1_0 + C1)/D)^2
            sc_sb = fin.tile([NT, TT], F32, tag="fin")
            nc.scalar.activation(sc_sb[:], s13_ps[64:64 + NT, :], AF.Copy)
            s3_sb = fin.tile([NT, TT], F32, tag="fin")
            nc.scalar.activation(s3_sb[:], s13_ps[32:32 + NT, :], AF.Copy)
            s2c = fin.tile([NT, TT], F32, tag="fin")
            nc.vector.scalar_tensor_tensor(
                s2c[:], sc_sb[:], 2.0, s2_ps[:], OP.mult, OP.add)
            mu2 = fin.tile([NT, TT], F32, tag="fin")
            nc.scalar.activation(mu2[:], s13_ps[0:NT, :], AF.Square,
                                 bias=epi_sb[:, 3:4], scale=1.0 / D)
            v = fin.tile([NT, TT], F32, tag="fin")
            nc.vector.scalar_tensor_tensor(
                v[:], s2c[:], 1.0 / D, mu2[:], OP.mult, OP.subtract)
            rv = fin.tile([NT, TT], F32, tag="fin")
            _act_direct(nc, rv[:], v[:], AF.Rsqrt, bias=epi_sb[:, 2:3])
            q = fin.tile([NT, TT], F32, tag="fin")
            nc.vector.scalar_tensor_tensor(
                q[:], s13_ps[0:NT, :], epi_sb[:, 1:2], s3_sb[:],
                OP.mult, OP.subtract)
            z = fin.tile([NT, TT], F32, tag="fin")
            nc.vector.scalar_tensor_tensor(
                z[:], q[:], epi_sb[:, 4:5], rv[:], OP.add, OP.mult)
            o_sb = fin.tile([NT, TT], F32, tag="fin")
            nc.scalar.activation(o_sb[:], z[:], AF.Identity,
                                 bias=epi_sb[:, 0:1], scale=-1.0)
            nc.sync.dma_start(out=out_d[:], in_=o_sb[:])

    _split_excess_waits(nc)
    return nc


_NC_CACHE = None


def _get_nc():
    global _NC_CACHE
    if _NC_CACHE is None:
        _NC_CACHE = _build_nc()
    return _NC_CACHE


def _host_prep(inputs):
    x = np.asarray(inputs["x"], dtype=np.float32)
    Ws = [np.asarray(inputs[n], np.float32) for n in
          ("Wf0", "Wi0", "Wh0", "Wf1", "Wi1", "Wh1")]
    bs = [np.asarray(inputs[n], np.float32) for n in
          ("bf0", "bi0", "bh0", "bf1", "bi1", "bh1")]
    bf0, bi0, bh0, bf1, bi1, bh1 = bs
    Wf1, Wi1, Wh1 = Ws[3], Ws[4], Ws[5]
    # h-bias elimination: layer-2 gate biases absorb Wx1 @ bh0
    bf1e = bf1 + Wf1 @ bh0
    bi1e = bi1 + Wi1 @ bh0
    bh1e = bh1 + Wh1 @ bh0

    # bf16 h-gate weights only
    wt_all = np.ascontiguousarray(
        np.stack([Ws[2].T, Ws[5].T])).astype(ml_dtypes.bfloat16)  # [2, din, dout]
    # fp8 f/i weights, scaled by WS (undone via the sigmoid input scale) to
    # keep them out of the fp8 subnormal range; [l*2+gate, p, c, i, dout]
    np_fp8 = mybir.dt.np(mybir.dt.float8e4)
    wfi = np.stack([
        (Ws[j].T * WS).reshape(2, 2, 128, D).transpose(2, 0, 1, 3)
        for j in (0, 1, 3, 4)
    ]).astype(np_fp8)

    bias = np.zeros((128, 2, 8), np.float32)
    gi = np.zeros((128, 2, G), np.float32)
    for g in range(G):
        sl = slice(g * 128, (g + 1) * 128)
        bias[:, 0, 0 * 4 + g] = bf0[sl]
        bias[:, 0, 1 * 4 + g] = bi0[sl]
        bias[:, 1, 0 * 4 + g] = bf1e[sl]
        bias[:, 1, 1 * 4 + g] = bi1e[sl]
        gi[:, 0, g] = -bh0[sl]
        gi[:, 1, g] = -bh1e[sl]
    # min-gate trick requires |bias| < 0.5 <= r = 1/(f+i)
    assert np.abs(bias).max() < 0.45, "bias magnitude breaks min-gate trick"

    w_out = np.asarray(inputs["W_out"], np.float32).reshape(D)
    ln_g = np.asarray(inputs["ln_g"], np.float32)
    ln_b = np.asarray(inputs["ln_b"], np.float32)
    b_out = float(np.asarray(inputs["b_out"], np.float32).reshape(()))
    wg = w_out * ln_g
    c = bh1e                         # constant channel shift of res
    c0 = float(w_out @ ln_b) + b_out
    swg = float(wg.sum())
    C1 = float(c.sum())
    C2 = float((c * c).sum())
    C3 = float((wg * c).sum())
    A = swg / D
    Kc = C1 * A - C3
    epsP = LN_EPS + C2 / D
    C1D = C1 / D

    slt = np.zeros((G, NT, 128, 72), np.float32)
    for g in range(G):
        sl = slice(g * 128, (g + 1) * 128)
        for t in range(NT):
            slt[g, t, :, t] = 1.0
            slt[g, t, :, 32 + t] = wg[sl]
            slt[g, t, :, 64 + t] = c[sl]
    slt = slt.astype(ml_dtypes.bfloat16)

    epi = np.zeros((NT, 8), np.float32)
    epi[:, 0] = c0
    epi[:, 1] = A
    epi[:, 2] = epsP
    epi[:, 3] = C1D
    epi[:, 4] = Kc

    xt = x.transpose(0, 2, 1)                              # [B, D, T]
    xt_b = np.ascontiguousarray(xt).astype(ml_dtypes.bfloat16)
    # fp8 x chunks for DoubleRow: [B, c, p, i, t], channel = c*256+i*128+p
    xf8 = np.ascontiguousarray(
        xt.reshape(B, 2, 2, 128, T).transpose(0, 1, 3, 2, 4)).astype(np_fp8)
    return xt_b, xf8, wt_all, wfi, bias, gi, slt, epi


def _in_maps(inputs):
    xt_b, xf8, wt_all, wfi, bias, gi, slt, epi = _host_prep(inputs)
    return [
        {
            "xt": xt_b[b], "xf8": xf8[b],
            "wt": wt_all, "wfi": wfi, "bias": bias, "gi": gi,
            "slt": slt, "epi": epi,
        }
        for b in range(B)
    ]


def kernel(**inputs):
    nc = _get_nc()
    res = run_bass_kernel_spmd(nc, _in_maps(inputs), list(range(B)))
    out = np.stack([res.results[b]["out"].reshape(T, OUT) for b in range(B)])
    return out.astype(np.float32)


def kernel_traced(**inputs):
    """same as kernel() but returns (output, BassKernelResults) with timing"""
    nc = _get_nc()
    res = run_bass_kernel_spmd(nc, _in_maps(inputs), list(range(B)), trace=True)
    out = np.stack([res.results[b]["out"].reshape(T, OUT) for b in range(B)])
    return out.astype(np.float32), res
